# revision 24
# baseline (speedup 1.0000x reference)
"""Causal single-head attention (B=2, S=4096, D=1024) + RoPE on 8 TRN2 cores.

Collective-free design: cores 4b+i (b=batch, i=rank 0..3) each receive the
FULL weights and the full x^T of their batch from the host, so no weight or
KV AllGather is needed.  Each core:

  1. projects+ropes Q^T for its own 8 q-chunks QCH[i] (balanced causal mass),
  2. projects K^T directly in transposed layout (stationary=W^T, moving=x^T)
     for ALL 4096 rows and ropes it in transposed layout,
  3. projects V naturally for all rows, interleaved with
  4. 8 attention slots: slot t attends the first 512*(t+1) kv columns; the
     causal edge mask is folded into the scores PSUM accumulation via one
     matmul (stationary=identity, moving=host-built mask), softmax runs
     without max-subtraction (|scores*scale| <= ~3, f32-exact), and exp reads
     PSUM tiles directly with accum_out partial row sums.

Output: final softmax-normalized rows (bf16); host scatters them.
"""

import sys

sys.path.insert(0, "/opt/trn_rl_repo")

import math
from contextlib import ExitStack

import ml_dtypes
import numpy as np

import concourse.bass as bass
import concourse.tile as tile
from concourse import bacc, mybir
from concourse.bass_utils import run_bass_kernel_spmd
from concourse.masks import make_identity

BF16 = mybir.dt.bfloat16
F32 = mybir.dt.float32
NPBF16 = ml_dtypes.bfloat16

B, S, D = 2, 4096, 1024
H = D // 2
C = 128
NQC = S // C                  # 32 chunks of 128 rows
NOC = 8                       # own q chunks per core
NOR = NOC * C                 # 1024 own q rows
NBLK = S // 512               # 8 512-row blocks
SCALE = 1.0 / math.sqrt(D)
NEG = -30000.0

QCH = [sorted([4 * t + i for t in range(4)] + [4 * t + 3 - i for t in range(4, 8)])
       for i in range(4)]

_CACHE = {}


def _emit_qchunk(nc, qstr_p, qraw_p, ktmp_p, qps_p, csq, wq_sb, xq_sb, qt_sb, blk):
    """Project (transposed layout) + rope own q chunk #blk."""
    qcols = slice(blk * C, (blk + 1) * C)
    cq = qstr_p.tile([C, 8, C], BF16, tag="cq", name=f"cq_{blk}")
    nc.sync.dma_start(cq[:], csq[:, :, qcols])
    qraw_c = qraw_p.tile([C, 8, C], BF16, tag="qraw", name=f"qraw_{blk}")
    for ec in range(8):
        qp = qps_p.tile([C, C], F32, tag="qps", name=f"qp_{blk}_{ec}")
        for dc in range(8):
            nc.tensor.matmul(qp[:], wq_sb[:, dc, ec * C:(ec + 1) * C],
                             xq_sb[:, dc, qcols],
                             start=(dc == 0), stop=(dc == 7))
        nc.scalar.copy(qraw_c[:, ec, :], qp[:])
    for pr in range(4):
        cc, ss = cq[:, pr, :], cq[:, pr + 4, :]
        re, im = qraw_c[:, pr, :], qraw_c[:, pr + 4, :]
        t0 = ktmp_p.tile([C, C], BF16, tag="qt0", name=f"qt0_{blk}_{pr}")
        t1 = ktmp_p.tile([C, C], BF16, tag="qt1", name=f"qt1_{blk}_{pr}")
        nc.vector.tensor_mul(t0[:], re, cc)
        nc.vector.tensor_mul(t1[:], im, ss)
        nc.vector.tensor_sub(qt_sb[:, pr, qcols], t0[:], t1[:])
        t2 = ktmp_p.tile([C, C], BF16, tag="qt2", name=f"qt2_{blk}_{pr}")
        t3 = ktmp_p.tile([C, C], BF16, tag="qt3", name=f"qt3_{blk}_{pr}")
        nc.vector.tensor_mul(t2[:], re, ss)
        nc.vector.tensor_mul(t3[:], im, cc)
        nc.vector.tensor_add(qt_sb[:, pr + 4, qcols], t2[:], t3[:])


def _build():
    nc = bacc.Bacc("TRN2", target_bir_lowering=False, debug=False,
                   enable_asserts=False, num_devices=8)

    xt = nc.dram_tensor("xt", [C, 8, S], BF16, kind="ExternalInput").ap()
    xq = nc.dram_tensor("xq", [C, 8, NOR], BF16, kind="ExternalInput").ap()
    csq = nc.dram_tensor("csq", [C, 8, NOR], BF16, kind="ExternalInput").ap()
    csk = nc.dram_tensor("csk", [C, 8, S], BF16, kind="ExternalInput").ap()
    w_in = nc.dram_tensor("w_in", [C, 24, D], BF16, kind="ExternalInput").ap()
    masks = nc.dram_tensor("masks", [C, 2, 512], BF16, kind="ExternalInput").ap()

    o_fin = nc.dram_tensor("o_fin", [NOC, C, D], BF16, kind="ExternalOutput").ap()

    with tile.TileContext(nc) as tc, ExitStack() as ctx:
        const_p = ctx.enter_context(tc.tile_pool(name="const", bufs=1))
        ident = const_p.tile([C, C], BF16)
        make_identity(nc, ident[:])
        masks_sb = const_p.tile([C, 2, 512], BF16)
        nc.sync.dma_start(masks_sb[:], masks)

        qt_p = ctx.enter_context(tc.tile_pool(name="qt", bufs=1))
        qt_sb = qt_p.tile([C, 8, NOR], BF16, tag="qt")
        # resident until the end: K^T and the V-projection weights (prefetched)
        kt_p = ctx.enter_context(tc.tile_pool(name="kt", bufs=1))
        kt_sb = kt_p.tile([C, 8, S], BF16, tag="kt")
        wv_sb = kt_p.tile([C, 8, D], BF16, tag="wv")

        # ---- phase 1: streamed K^T projection+rope for all rows, with the
        #      own-q chunk of each block projected+roped along the way ----
        with tc.tile_pool(name="ph1", bufs=1) as p1_p, \
             tc.tile_pool(name="kstr", bufs=2) as kstr_p, \
             tc.tile_pool(name="cstr", bufs=2) as cstr_p, \
             tc.tile_pool(name="qstr", bufs=2) as qstr_p, \
             tc.tile_pool(name="kraw", bufs=4) as kraw_p, \
             tc.tile_pool(name="qraw", bufs=2) as qraw_p, \
             tc.tile_pool(name="ktmp", bufs=2) as ktmp_p, \
             tc.tile_pool(name="kps", bufs=4, space="PSUM") as kps_p, \
             tc.tile_pool(name="qps", bufs=2, space="PSUM") as qps_p:
            # DMA issue order = order of first use: Q chunk 0 work (wq + a
            # 2KB xq slice) starts the PE while the big K-phase DMAs stream.
            wq_sb = p1_p.tile([C, 8, D], BF16, tag="wq")
            nc.sync.dma_start(wq_sb[:], w_in[:, 0:8, :])
            xq_sb = p1_p.tile([C, 8, NOR], BF16, tag="xq")
            nc.sync.dma_start(xq_sb[:, :, 0:C], xq[:, :, 0:C])
            wk_sb = p1_p.tile([C, 8, D], BF16, tag="wk")
            nc.sync.dma_start(wk_sb[:], w_in[:, 8:16, :])
            xb0 = kstr_p.tile([C, 8, 512], BF16, tag="xb", name="xb_0")
            nc.sync.dma_start(xb0[:], xt[:, :, 0:512])
            cb0 = cstr_p.tile([C, 8, 512], BF16, tag="cb", name="cb_0")
            nc.sync.dma_start(cb0[:], csk[:, :, 0:512])
            nc.sync.dma_start(xq_sb[:, :, C:NOR], xq[:, :, C:NOR])
            nc.sync.dma_start(wv_sb[:], w_in[:, 16:24, :])
            for blk in range(NBLK):
                rows = slice(blk * 512, (blk + 1) * 512)
                if blk == 0:
                    xb, cb = xb0, cb0
                else:
                    xb = kstr_p.tile([C, 8, 512], BF16, tag="xb", name=f"xb_{blk}")
                    nc.sync.dma_start(xb[:], xt[:, :, rows])
                    cb = cstr_p.tile([C, 8, 512], BF16, tag="cb", name=f"cb_{blk}")
                    nc.sync.dma_start(cb[:], csk[:, :, rows])
                if blk == 0:
                    _emit_qchunk(nc, qstr_p, qraw_p, ktmp_p, qps_p, csq,
                                 wq_sb, xq_sb, qt_sb, blk)
                for pr in range(4):
                    kraw = []
                    for dc in (pr, pr + 4):
                        ps = kps_p.tile([C, 512], F32, tag="kps",
                                        name=f"kps_{blk}_{dc}")
                        for dcd in range(8):
                            nc.tensor.matmul(ps[:],
                                             wk_sb[:, dcd, dc * C:(dc + 1) * C],
                                             xb[:, dcd, :],
                                             start=(dcd == 0), stop=(dcd == 7))
                        kr = kraw_p.tile([C, 512], BF16, tag="kraw",
                                         name=f"kraw_{blk}_{dc}")
                        nc.scalar.copy(kr[:], ps[:])
                        kraw.append(kr)
                    re, im = kraw[0], kraw[1]
                    cc, ss = cb[:, pr, :], cb[:, pr + 4, :]
                    t0 = ktmp_p.tile([C, 512], BF16, tag="kt0", name=f"kt0_{blk}_{pr}")
                    t1 = ktmp_p.tile([C, 512], BF16, tag="kt1", name=f"kt1_{blk}_{pr}")
                    nc.vector.tensor_mul(t0[:], re[:], cc)
                    nc.vector.tensor_mul(t1[:], im[:], ss)
                    nc.vector.tensor_sub(kt_sb[:, pr, rows], t0[:], t1[:])
                    t2 = ktmp_p.tile([C, 512], BF16, tag="kt2", name=f"kt2_{blk}_{pr}")
                    t3 = ktmp_p.tile([C, 512], BF16, tag="kt3", name=f"kt3_{blk}_{pr}")
                    nc.vector.tensor_mul(t2[:], re[:], ss)
                    nc.vector.tensor_mul(t3[:], im[:], cc)
                    nc.vector.tensor_add(kt_sb[:, pr + 4, rows], t2[:], t3[:])

                if blk > 0:
                    _emit_qchunk(nc, qstr_p, qraw_p, ktmp_p, qps_p, csq,
                                 wq_sb, xq_sb, qt_sb, blk)

        # ---- phase V + attention, interleaved per 512-block/slot ----
        v_p = ctx.enter_context(tc.tile_pool(name="v", bufs=1))
        v_sb = v_p.tile([C, NQC, D], BF16, tag="v")
        with tc.tile_pool(name="vstr", bufs=2) as vstr_p, \
             tc.tile_pool(name="pp", bufs=2) as pp_p, \
             tc.tile_pool(name="at", bufs=2) as at_p, \
             tc.tile_pool(name="ptp", bufs=9) as pt_p, \
             tc.tile_pool(name="vps", bufs=2, space="PSUM") as vps_p, \
             tc.tile_pool(name="scps", bufs=2, space="PSUM") as scps_p, \
             tc.tile_pool(name="ops", bufs=1, space="PSUM") as ops_p, \
             tc.tile_pool(name="ptps", bufs=2, space="PSUM") as ptps_p:
            for t in range(NOC):
                # V projection for kv block t (chunks 4t..4t+3)
                rows = slice(t * 512, (t + 1) * 512)
                xb = vstr_p.tile([C, 8, 512], BF16, tag="vxb", name=f"vxb_{t}")
                nc.sync.dma_start(xb[:], xt[:, :, rows])
                for c4 in range(4):
                    rsl = slice(c4 * C, (c4 + 1) * C)
                    for h in range(2):
                        cols = slice(h * 512, (h + 1) * 512)
                        vp = vps_p.tile([C, 512], F32, tag="vps",
                                        name=f"vps_{t}_{c4}_{h}")
                        for dc in range(8):
                            nc.tensor.matmul(vp[:], xb[:, dc, rsl],
                                             wv_sb[:, dc, cols],
                                             start=(dc == 0), stop=(dc == 7))
                        nc.scalar.copy(v_sb[:, 4 * t + c4, cols], vp[:])

                # attention slot t: q chunk QCH[i][t], kv cols [0, 512*(t+1))
                kpat = 0 if t < 4 else 1
                qc = slice(t * C, (t + 1) * C)
                W = 512 * (t + 1)
                p_sb = pp_p.tile([C, S], BF16, tag="p", name=f"p_{t}")
                lsump = at_p.tile([C, NOC], F32, tag="lsump", name=f"lsump_{t}")
                for u in range(t + 1):
                    cols = slice(u * 512, (u + 1) * 512)
                    sps = scps_p.tile([C, 512], F32, tag="scps", name=f"sps_{t}_{u}")
                    for dc in range(8):
                        nc.tensor.matmul(sps[:], qt_sb[:, dc, qc],
                                         kt_sb[:, dc, cols],
                                         start=(dc == 0),
                                         stop=(dc == 7 and u != t))
                    if u == t:
                        nc.tensor.matmul(sps[:], ident[:], masks_sb[:, kpat, :],
                                         start=False, stop=True)
                    nc.scalar.activation(p_sb[:, cols], sps[:],
                                         mybir.ActivationFunctionType.Exp,
                                         scale=SCALE,
                                         accum_out=lsump[:, u:u + 1])
                lsum = at_p.tile([C, 1], F32, tag="lsum", name=f"lsum_{t}")
                nc.vector.tensor_reduce(lsum[:], lsump[:, 0:t + 1],
                                        axis=mybir.AxisListType.X,
                                        op=mybir.AluOpType.add)
                rinv = at_p.tile([C, 1], F32, tag="rinv", name=f"rinv_{t}")
                nc.vector.reciprocal(rinv[:], lsum[:])

                o_ps = ops_p.tile([C, D], F32, tag="ops", name=f"ops_{t}")
                ob = at_p.tile([C, D], BF16, tag="ob", name=f"ob_{t}")
                pts = []
                for u in range(t + 1):
                    ptp = ptps_p.tile([C, 512], BF16, tag="ptps", name=f"ptp_{t}_{u}")
                    for j in range(4):
                        nc.tensor.transpose(ptp[:, j * C:(j + 1) * C],
                                            p_sb[:, (4 * u + j) * C:(4 * u + j + 1) * C],
                                            ident[:])
                    pt_sb = pt_p.tile([C, 512], BF16, tag="pt", name=f"pt_{t}_{u}")
                    nc.vector.tensor_copy(pt_sb[:], ptp[:])
                    pts.append(pt_sb)
                for h in range(2):
                    cols = slice(h * 512, (h + 1) * 512)
                    for u in range(t + 1):
                        for j in range(4):
                            nc.tensor.matmul(o_ps[:, cols],
                                             pts[u][:, j * C:(j + 1) * C],
                                             v_sb[:, 4 * u + j, cols],
                                             start=(u == 0 and j == 0),
                                             stop=(u == t and j == 3))
                    nc.vector.tensor_scalar_mul(ob[:, cols], o_ps[:, cols],
                                                rinv[:])
                    nc.sync.dma_start(o_fin[t, :, cols], ob[:, cols])

    nc.compile()
    return nc


def _xt_blocked(rows_x):
    """[n, D] float -> [C, 8, n] bf16 blocked transpose."""
    return np.ascontiguousarray(
        rows_x.astype(NPBF16).reshape(-1, 8, C).transpose(2, 1, 0))


def _masks(i):
    tri = np.where(np.arange(C)[:, None] >= np.arange(C)[None, :], 0.0, NEG)
    m = np.zeros((C, 2, 512), np.float32)
    for k, diag in enumerate((i, 3 - i)):
        for c in range(4):
            if c > diag:
                m[:, k, c * C:(c + 1) * C] = NEG
            elif c == diag:
                m[:, k, c * C:(c + 1) * C] = tri
    return np.ascontiguousarray(m.astype(NPBF16))


def _prep_inputs(x, w_q, w_k, w_v, freqs_cos, freqs_sin):
    perm = np.concatenate([np.arange(0, D, 2), np.arange(1, D, 2)])
    wqT = np.ascontiguousarray(w_q[perm, :].T)
    wkT = np.ascontiguousarray(w_k[perm, :].T)
    wvT = np.ascontiguousarray(w_v.T)

    def blk(wt):  # [D, D] -> [C, dc, e]
        return wt.astype(NPBF16).reshape(8, C, D).transpose(1, 0, 2)

    flat24 = np.ascontiguousarray(
        np.concatenate([blk(wqT), blk(wkT), blk(wvT)], axis=1))  # [C, 24, D]

    cs_all = np.concatenate([freqs_cos, freqs_sin], axis=1)  # [S, D]
    csk_b = _xt_blocked(cs_all)                              # same for all cores
    xt_b = [_xt_blocked(np.asarray(x[b])) for b in range(B)]

    in_maps = []
    for core in range(8):
        b, i = divmod(core, 4)
        qrows = (np.asarray(QCH[i])[:, None] * C + np.arange(C)[None, :]).reshape(-1)
        in_maps.append({
            "xt": xt_b[b],
            "xq": _xt_blocked(np.asarray(x[b])[qrows]),
            "csq": _xt_blocked(cs_all[qrows]),
            "csk": csk_b,
            "w_in": flat24,
            "masks": _masks(i),
        })
    return in_maps


def _assemble(results):
    out = np.empty((B, S, D), np.float32)
    for core in range(8):
        b, i = divmod(core, 4)
        o = np.asarray(results[core]["o_fin"], np.float32)  # [NOC, C, D]
        for t, j in enumerate(QCH[i]):
            out[b, j * C:(j + 1) * C] = o[t]
    return out


def _run_pjrt(nc, in_maps, n_cores=8):
    """Like bass2jax.run_bass_via_pjrt, but creates the donated output
    buffers ON DEVICE (jit zeros) instead of uploading host zeros."""
    import jax
    import jax.numpy as jnp
    from jax.sharding import Mesh, NamedSharding, PartitionSpec
    try:
        from jax import shard_map
    except ImportError:
        from jax.experimental.shard_map import shard_map
    from concourse.bass2jax import (_bass_exec_p, install_neuronx_cc_hook,
                                    partition_id_tensor)

    install_neuronx_cc_hook()
    partition_name = nc.partition_id_tensor.name if nc.partition_id_tensor else None
    in_names, out_names, out_avals = [], [], []
    for alloc in nc.m.functions[0].allocations:
        if not isinstance(alloc, mybir.MemoryLocationSet):
            continue
        name = alloc.memorylocations[0].name
        if alloc.kind == "ExternalInput":
            if name != partition_name:
                in_names.append(name)
        elif alloc.kind == "ExternalOutput":
            out_avals.append(jax.core.ShapedArray(
                tuple(alloc.tensor_shape), mybir.dt.np(alloc.dtype)))
            out_names.append(name)
    n_params = len(in_names)
    all_in = list(in_names) + list(out_names)
    if partition_name is not None:
        all_in.append(partition_name)
    donate = tuple(range(n_params, n_params + len(out_names)))

    def _body(*args):
        operands = list(args)
        if partition_name is not None:
            operands.append(partition_id_tensor())
        return tuple(_bass_exec_p.bind(
            *operands, out_avals=tuple(out_avals), in_names=tuple(all_in),
            out_names=tuple(out_names), lowering_input_output_aliases=(),
            sim_require_finite=True, sim_require_nnan=True, nc=nc))

    devices = jax.devices()[:n_cores]
    mesh = Mesh(np.asarray(devices), ("core",))
    nio = n_params + len(out_names)
    sm_kw = dict(mesh=mesh, in_specs=(PartitionSpec("core"),) * nio,
                 out_specs=(PartitionSpec("core"),) * len(out_names))
    try:
        smapped = shard_map(_body, check_vma=False, **sm_kw)
    except TypeError:
        smapped = shard_map(_body, check_rep=False, **sm_kw)
    sharded = jax.jit(smapped, donate_argnums=donate, keep_unused=True)
    sh = NamedSharding(mesh, PartitionSpec("core"))
    zeros = jax.jit(
        lambda: tuple(jnp.zeros((n_cores * a.shape[0], *a.shape[1:]), a.dtype)
                      for a in out_avals),
        out_shardings=(sh,) * len(out_avals))()
    concat_in = [np.concatenate([np.asarray(m[k]) for m in in_maps], axis=0)
                 for k in in_names]
    outs = [np.asarray(o) for o in sharded(*concat_in, *zeros)]
    per_core = []
    for c in range(n_cores):
        d = {}
        for name, arr in zip(out_names, outs):
            s0 = arr.shape[0] // n_cores
            d[name] = arr[c * s0:(c + 1) * s0]
        per_core.append(d)
    return per_core


def kernel(x, w_q, w_k, w_v, freqs_cos, freqs_sin, _want_results=False, _trace=False):
    if "nc" not in _CACHE:
        _CACHE["nc"] = _build()
    nc = _CACHE["nc"]
    in_maps = _prep_inputs(np.asarray(x, np.float32), np.asarray(w_q, np.float32),
                           np.asarray(w_k, np.float32), np.asarray(w_v, np.float32),
                           np.asarray(freqs_cos, np.float32),
                           np.asarray(freqs_sin, np.float32))
    if _trace:
        kr = run_bass_kernel_spmd(nc, in_maps, core_ids=list(range(8)), trace=True)
        out = _assemble(kr.results)
        return (out, kr) if _want_results else out
    try:
        results = _run_pjrt(nc, in_maps)
    except Exception as e:
        print(f"kernel: _run_pjrt failed ({type(e).__name__}: {e}); "
              "falling back to run_bass_kernel_spmd", file=sys.stderr)
        kr = run_bass_kernel_spmd(nc, in_maps, core_ids=list(range(8)))
        results = kr.results
    out = _assemble(results)
    if _want_results:
        return out, results
    return out


# revision 35
# speedup vs baseline: 1.0141x; 1.0141x over previous
"""Causal single-head attention (B=2, S=4096, D=1024) + RoPE on 8 TRN2 cores.

Collective-free design: cores 4b+i (b=batch, i=rank 0..3) each receive the
FULL weights and the full x^T of their batch from the host, so no weight or
KV AllGather is needed.  Each core:

  1. projects+ropes Q^T for its own 8 q-chunks QCH[i] (balanced causal mass),
  2. projects K^T directly in transposed layout (stationary=W^T, moving=x^T)
     for ALL 4096 rows and ropes it in transposed layout,
  3. projects V naturally for all rows, interleaved with
  4. 8 attention slots: slot t attends the first 512*(t+1) kv columns; the
     causal edge mask is folded into the scores PSUM accumulation via one
     matmul (stationary=identity, moving=host-built mask), softmax runs
     without max-subtraction (|scores*scale| <= ~3, f32-exact), and exp reads
     PSUM tiles directly with accum_out partial row sums.

Output: final softmax-normalized rows (bf16); host scatters them.
"""

import sys

sys.path.insert(0, "/opt/trn_rl_repo")

import math
from contextlib import ExitStack

import ml_dtypes
import numpy as np

import concourse.bass as bass
import concourse.tile as tile
from concourse import bacc, mybir
from concourse.bass_utils import run_bass_kernel_spmd
from concourse.masks import make_identity

BF16 = mybir.dt.bfloat16
F32 = mybir.dt.float32
NPBF16 = ml_dtypes.bfloat16

B, S, D = 2, 4096, 1024
H = D // 2
C = 128
NQC = S // C                  # 32 chunks of 128 rows
NOC = 8                       # own q chunks per core
NOR = NOC * C                 # 1024 own q rows
NBLK = S // 512               # 8 512-row blocks
SCALE = 1.0 / math.sqrt(D)
NEG = -30000.0

QCH = [sorted([4 * t + i for t in range(4)] + [4 * t + 3 - i for t in range(4, 8)])
       for i in range(4)]

_CACHE = {}


def _emit_qchunk(nc, qstr_p, qraw_p, ktmp_p, qps_p, csq, wq_sb, xq_sb, qt_sb, blk):
    """Project (transposed layout) + rope own q chunk #blk."""
    qcols = slice(blk * C, (blk + 1) * C)
    cq = qstr_p.tile([C, 8, C], BF16, tag="cq", name=f"cq_{blk}")
    nc.sync.dma_start(cq[:], csq[:, :, qcols])
    qraw_c = qraw_p.tile([C, 8, C], BF16, tag="qraw", name=f"qraw_{blk}")
    for ec in range(8):
        qp = qps_p.tile([C, C], F32, tag="qps", name=f"qp_{blk}_{ec}")
        for dc in range(8):
            nc.tensor.matmul(qp[:], wq_sb[:, dc, ec * C:(ec + 1) * C],
                             xq_sb[:, dc, qcols],
                             start=(dc == 0), stop=(dc == 7))
        nc.scalar.copy(qraw_c[:, ec, :], qp[:])
    for pr in range(4):
        cc, ss = cq[:, pr, :], cq[:, pr + 4, :]
        re, im = qraw_c[:, pr, :], qraw_c[:, pr + 4, :]
        t0 = ktmp_p.tile([C, C], BF16, tag="qt0", name=f"qt0_{blk}_{pr}")
        t1 = ktmp_p.tile([C, C], BF16, tag="qt1", name=f"qt1_{blk}_{pr}")
        nc.vector.tensor_mul(t0[:], re, cc)
        nc.vector.tensor_mul(t1[:], im, ss)
        nc.vector.tensor_sub(qt_sb[:, pr, qcols], t0[:], t1[:])
        t2 = ktmp_p.tile([C, C], BF16, tag="qt2", name=f"qt2_{blk}_{pr}")
        t3 = ktmp_p.tile([C, C], BF16, tag="qt3", name=f"qt3_{blk}_{pr}")
        nc.vector.tensor_mul(t2[:], re, ss)
        nc.vector.tensor_mul(t3[:], im, cc)
        nc.vector.tensor_add(qt_sb[:, pr + 4, qcols], t2[:], t3[:])


def _build():
    nc = bacc.Bacc("TRN2", target_bir_lowering=False, debug=False,
                   enable_asserts=False, num_devices=8)

    xt = nc.dram_tensor("xt", [C, 8, S], BF16, kind="ExternalInput").ap()
    xq = nc.dram_tensor("xq", [C, 8, NOR], BF16, kind="ExternalInput").ap()
    csq = nc.dram_tensor("csq", [C, 8, NOR], BF16, kind="ExternalInput").ap()
    csk = nc.dram_tensor("csk", [C, 8, S], BF16, kind="ExternalInput").ap()
    w_in = nc.dram_tensor("w_in", [C, 24, D], BF16, kind="ExternalInput").ap()
    masks = nc.dram_tensor("masks", [C, 2, 512], BF16, kind="ExternalInput").ap()

    o_fin = nc.dram_tensor("o_fin", [NOC, C, D], BF16, kind="ExternalOutput").ap()

    with tile.TileContext(nc) as tc, ExitStack() as ctx:
        const_p = ctx.enter_context(tc.tile_pool(name="const", bufs=1))
        ident = const_p.tile([C, C], BF16)
        make_identity(nc, ident[:])
        masks_sb = const_p.tile([C, 2, 512], BF16)
        nc.sync.dma_start(masks_sb[:], masks)

        # PE warmup: ~11us of dummy transposes while the first weight/x DMAs
        # stream, so the tensor engine is at full p-state when real matmuls
        # start (the results are never read). Source is a fast DVE memset
        # tile so warmup starts before the Pool-built identity is ready.
        with tc.tile_pool(name="wup", bufs=1, space="PSUM") as wup_p, \
             tc.tile_pool(name="wsrc", bufs=1) as wsrc_p:
            wsrc = wsrc_p.tile([C, C], BF16, tag="wsrc")
            nc.vector.memset(wsrc[:], 0.0)
            wup = wup_p.tile([C, C], BF16, tag="wup")
            for r in range(185):
                nc.tensor.transpose(wup[:], wsrc[:], wsrc[:])

        qt_p = ctx.enter_context(tc.tile_pool(name="qt", bufs=1))
        qt_sb = qt_p.tile([C, 8, NOR], BF16, tag="qt")
        # resident until the end: K^T and the V-projection weights (prefetched)
        kt_p = ctx.enter_context(tc.tile_pool(name="kt", bufs=1))
        kt_sb = kt_p.tile([C, 8, S], BF16, tag="kt")
        wv_sb = kt_p.tile([C, 8, D], BF16, tag="wv")

        # ---- phase 1: streamed K^T projection+rope for all rows, with the
        #      own-q chunk of each block projected+roped along the way ----
        with tc.tile_pool(name="ph1", bufs=1) as p1_p, \
             tc.tile_pool(name="kstr", bufs=2) as kstr_p, \
             tc.tile_pool(name="cstr", bufs=2) as cstr_p, \
             tc.tile_pool(name="qstr", bufs=2) as qstr_p, \
             tc.tile_pool(name="kraw", bufs=4) as kraw_p, \
             tc.tile_pool(name="qraw", bufs=2) as qraw_p, \
             tc.tile_pool(name="ktmp", bufs=2) as ktmp_p, \
             tc.tile_pool(name="kps", bufs=4, space="PSUM") as kps_p, \
             tc.tile_pool(name="qps", bufs=3, space="PSUM") as qps_p:
            # DMA issue order = order of first use: K block 0 only needs
            # wk+xb0 (8.7us of DMA) -> PE starts at ~11us; Q-chunk inputs
            # stream while K block 0 runs and fill later block boundaries.
            wk_sb = p1_p.tile([C, 8, D], BF16, tag="wk")
            nc.sync.dma_start(wk_sb[:], w_in[:, 8:16, :])
            xb0 = kstr_p.tile([C, 8, 512], BF16, tag="xb", name="xb_0")
            nc.sync.dma_start(xb0[:], xt[:, :, 0:512])
            cb0 = cstr_p.tile([C, 8, 512], BF16, tag="cb", name="cb_0")
            nc.sync.dma_start(cb0[:], csk[:, :, 0:512])
            wq_sb = p1_p.tile([C, 8, D], BF16, tag="wq")
            nc.sync.dma_start(wq_sb[:], w_in[:, 0:8, :])
            xq_sb = p1_p.tile([C, 8, NOR], BF16, tag="xq")
            nc.sync.dma_start(xq_sb[:, :, 0:2 * C], xq[:, :, 0:2 * C])
            for blk in range(NBLK):
                if blk == 1:
                    nc.sync.dma_start(xq_sb[:, :, 2 * C:NOR], xq[:, :, 2 * C:NOR])
                if blk == 2:
                    # wv needed only in phase V; issue behind the early x blocks
                    nc.sync.dma_start(wv_sb[:], w_in[:, 16:24, :])
                rows = slice(blk * 512, (blk + 1) * 512)
                if blk == 0:
                    xb, cb = xb0, cb0
                else:
                    xb = kstr_p.tile([C, 8, 512], BF16, tag="xb", name=f"xb_{blk}")
                    nc.sync.dma_start(xb[:], xt[:, :, rows])
                    cb = cstr_p.tile([C, 8, 512], BF16, tag="cb", name=f"cb_{blk}")
                    nc.sync.dma_start(cb[:], csk[:, :, rows])
                for pr in range(4):
                    kraw = []
                    for dc in (pr, pr + 4):
                        ps = kps_p.tile([C, 512], F32, tag="kps",
                                        name=f"kps_{blk}_{dc}")
                        for dcd in range(8):
                            nc.tensor.matmul(ps[:],
                                             wk_sb[:, dcd, dc * C:(dc + 1) * C],
                                             xb[:, dcd, :],
                                             start=(dcd == 0), stop=(dcd == 7))
                        kr = kraw_p.tile([C, 512], BF16, tag="kraw",
                                         name=f"kraw_{blk}_{dc}")
                        nc.scalar.copy(kr[:], ps[:])
                        kraw.append(kr)
                    re, im = kraw[0], kraw[1]
                    cc, ss = cb[:, pr, :], cb[:, pr + 4, :]
                    t0 = ktmp_p.tile([C, 512], BF16, tag="kt0", name=f"kt0_{blk}_{pr}")
                    t1 = ktmp_p.tile([C, 512], BF16, tag="kt1", name=f"kt1_{blk}_{pr}")
                    nc.vector.tensor_mul(t0[:], re[:], cc)
                    nc.vector.tensor_mul(t1[:], im[:], ss)
                    nc.vector.tensor_sub(kt_sb[:, pr, rows], t0[:], t1[:])
                    t2 = ktmp_p.tile([C, 512], BF16, tag="kt2", name=f"kt2_{blk}_{pr}")
                    t3 = ktmp_p.tile([C, 512], BF16, tag="kt3", name=f"kt3_{blk}_{pr}")
                    nc.vector.tensor_mul(t2[:], re[:], ss)
                    nc.vector.tensor_mul(t3[:], im[:], cc)
                    nc.vector.tensor_add(kt_sb[:, pr + 4, rows], t2[:], t3[:])

                _emit_qchunk(nc, qstr_p, qraw_p, ktmp_p, qps_p, csq,
                             wq_sb, xq_sb, qt_sb, blk)

        # ---- phase V + attention, interleaved per 512-block/slot ----
        v_p = ctx.enter_context(tc.tile_pool(name="v", bufs=1))
        v_sb = v_p.tile([C, NQC, D], BF16, tag="v")
        with tc.tile_pool(name="vstr", bufs=2) as vstr_p, \
             tc.tile_pool(name="pp", bufs=2) as pp_p, \
             tc.tile_pool(name="at", bufs=2) as at_p, \
             tc.tile_pool(name="ptp", bufs=9) as pt_p, \
             tc.tile_pool(name="vps", bufs=2, space="PSUM") as vps_p, \
             tc.tile_pool(name="scps", bufs=2, space="PSUM") as scps_p, \
             tc.tile_pool(name="ops", bufs=2, space="PSUM") as ops_p, \
             tc.tile_pool(name="ptps", bufs=2, space="PSUM") as ptps_p:
            for t in range(NOC):
                # V projection for kv block t (chunks 4t..4t+3)
                rows = slice(t * 512, (t + 1) * 512)
                xb = vstr_p.tile([C, 8, 512], BF16, tag="vxb", name=f"vxb_{t}")
                nc.sync.dma_start(xb[:], xt[:, :, rows])
                for c4 in range(4):
                    rsl = slice(c4 * C, (c4 + 1) * C)
                    for h in range(2):
                        cols = slice(h * 512, (h + 1) * 512)
                        vp = vps_p.tile([C, 512], F32, tag="vps",
                                        name=f"vps_{t}_{c4}_{h}")
                        for dc in range(8):
                            nc.tensor.matmul(vp[:], xb[:, dc, rsl],
                                             wv_sb[:, dc, cols],
                                             start=(dc == 0), stop=(dc == 7))
                        nc.scalar.copy(v_sb[:, 4 * t + c4, cols], vp[:])

                # attention slot t: q chunk QCH[i][t], kv cols [0, 512*(t+1))
                kpat = 0 if t < 4 else 1
                qc = slice(t * C, (t + 1) * C)
                W = 512 * (t + 1)
                p_sb = pp_p.tile([C, S], BF16, tag="p", name=f"p_{t}")
                lsump = at_p.tile([C, NOC], F32, tag="lsump", name=f"lsump_{t}")
                for u in range(t + 1):
                    cols = slice(u * 512, (u + 1) * 512)
                    sps = scps_p.tile([C, 512], F32, tag="scps", name=f"sps_{t}_{u}")
                    for dc in range(8):
                        nc.tensor.matmul(sps[:], qt_sb[:, dc, qc],
                                         kt_sb[:, dc, cols],
                                         start=(dc == 0),
                                         stop=(dc == 7 and u != t))
                    if u == t:
                        nc.tensor.matmul(sps[:], ident[:], masks_sb[:, kpat, :],
                                         start=False, stop=True)
                    nc.scalar.activation(p_sb[:, cols], sps[:],
                                         mybir.ActivationFunctionType.Exp,
                                         scale=SCALE,
                                         accum_out=lsump[:, u:u + 1])
                lsum = at_p.tile([C, 1], F32, tag="lsum", name=f"lsum_{t}")
                nc.vector.tensor_reduce(lsum[:], lsump[:, 0:t + 1],
                                        axis=mybir.AxisListType.X,
                                        op=mybir.AluOpType.add)
                rinv = at_p.tile([C, 1], F32, tag="rinv", name=f"rinv_{t}")
                nc.vector.reciprocal(rinv[:], lsum[:])

                ob = at_p.tile([C, D], BF16, tag="ob", name=f"ob_{t}")
                pts = []
                for u in range(t + 1):
                    ptp = ptps_p.tile([C, 512], BF16, tag="ptps", name=f"ptp_{t}_{u}")
                    for j in range(4):
                        nc.tensor.transpose(ptp[:, j * C:(j + 1) * C],
                                            p_sb[:, (4 * u + j) * C:(4 * u + j + 1) * C],
                                            ident[:])
                    pt_sb = pt_p.tile([C, 512], BF16, tag="pt", name=f"pt_{t}_{u}")
                    nc.vector.tensor_copy(pt_sb[:], ptp[:])
                    pts.append(pt_sb)
                for h in range(2):
                    cols = slice(h * 512, (h + 1) * 512)
                    o_ps = ops_p.tile([C, 512], F32, tag="ops",
                                      name=f"ops_{t}_{h}")
                    for u in range(t + 1):
                        for j in range(4):
                            nc.tensor.matmul(o_ps[:],
                                             pts[u][:, j * C:(j + 1) * C],
                                             v_sb[:, 4 * u + j, cols],
                                             start=(u == 0 and j == 0),
                                             stop=(u == t and j == 3))
                    if h == 0:
                        nc.vector.tensor_scalar_mul(ob[:, cols], o_ps[:],
                                                    rinv[:])
                    else:
                        nc.scalar.mul(ob[:, cols], o_ps[:], rinv[:])
                    nc.sync.dma_start(o_fin[t, :, cols], ob[:, cols])

    nc.compile()
    return nc


def _xt_blocked(rows_x):
    """[n, D] float -> [C, 8, n] bf16 blocked transpose."""
    return np.ascontiguousarray(
        rows_x.astype(NPBF16).reshape(-1, 8, C).transpose(2, 1, 0))


def _masks(i):
    tri = np.where(np.arange(C)[:, None] >= np.arange(C)[None, :], 0.0, NEG)
    m = np.zeros((C, 2, 512), np.float32)
    for k, diag in enumerate((i, 3 - i)):
        for c in range(4):
            if c > diag:
                m[:, k, c * C:(c + 1) * C] = NEG
            elif c == diag:
                m[:, k, c * C:(c + 1) * C] = tri
    return np.ascontiguousarray(m.astype(NPBF16))


def _prep_inputs(x, w_q, w_k, w_v, freqs_cos, freqs_sin):
    perm = np.concatenate([np.arange(0, D, 2), np.arange(1, D, 2)])
    wqT = np.ascontiguousarray(w_q[perm, :].T)
    wkT = np.ascontiguousarray(w_k[perm, :].T)
    wvT = np.ascontiguousarray(w_v.T)

    def blk(wt):  # [D, D] -> [C, dc, e]
        return wt.astype(NPBF16).reshape(8, C, D).transpose(1, 0, 2)

    flat24 = np.ascontiguousarray(
        np.concatenate([blk(wqT), blk(wkT), blk(wvT)], axis=1))  # [C, 24, D]

    cs_all = np.concatenate([freqs_cos, freqs_sin], axis=1)  # [S, D]
    csk_b = _xt_blocked(cs_all)                              # same for all cores
    xt_b = [_xt_blocked(np.asarray(x[b])) for b in range(B)]

    in_maps = []
    for core in range(8):
        b, i = divmod(core, 4)
        qrows = (np.asarray(QCH[i])[:, None] * C + np.arange(C)[None, :]).reshape(-1)
        in_maps.append({
            "xt": xt_b[b],
            "xq": _xt_blocked(np.asarray(x[b])[qrows]),
            "csq": _xt_blocked(cs_all[qrows]),
            "csk": csk_b,
            "w_in": flat24,
            "masks": _masks(i),
        })
    return in_maps


def _assemble(results):
    out = np.empty((B, S, D), np.float32)
    for core in range(8):
        b, i = divmod(core, 4)
        o = np.asarray(results[core]["o_fin"], np.float32)  # [NOC, C, D]
        for t, j in enumerate(QCH[i]):
            out[b, j * C:(j + 1) * C] = o[t]
    return out


def _run_pjrt(nc, in_maps, n_cores=8):
    """Like bass2jax.run_bass_via_pjrt, but creates the donated output
    buffers ON DEVICE (jit zeros) instead of uploading host zeros."""
    import jax
    import jax.numpy as jnp
    from jax.sharding import Mesh, NamedSharding, PartitionSpec
    try:
        from jax import shard_map
    except ImportError:
        from jax.experimental.shard_map import shard_map
    from concourse.bass2jax import (_bass_exec_p, install_neuronx_cc_hook,
                                    partition_id_tensor)

    install_neuronx_cc_hook()
    partition_name = nc.partition_id_tensor.name if nc.partition_id_tensor else None
    in_names, out_names, out_avals = [], [], []
    for alloc in nc.m.functions[0].allocations:
        if not isinstance(alloc, mybir.MemoryLocationSet):
            continue
        name = alloc.memorylocations[0].name
        if alloc.kind == "ExternalInput":
            if name != partition_name:
                in_names.append(name)
        elif alloc.kind == "ExternalOutput":
            out_avals.append(jax.core.ShapedArray(
                tuple(alloc.tensor_shape), mybir.dt.np(alloc.dtype)))
            out_names.append(name)
    n_params = len(in_names)
    all_in = list(in_names) + list(out_names)
    if partition_name is not None:
        all_in.append(partition_name)
    donate = tuple(range(n_params, n_params + len(out_names)))

    def _body(*args):
        operands = list(args)
        if partition_name is not None:
            operands.append(partition_id_tensor())
        return tuple(_bass_exec_p.bind(
            *operands, out_avals=tuple(out_avals), in_names=tuple(all_in),
            out_names=tuple(out_names), lowering_input_output_aliases=(),
            sim_require_finite=True, sim_require_nnan=True, nc=nc))

    devices = jax.devices()[:n_cores]
    mesh = Mesh(np.asarray(devices), ("core",))
    nio = n_params + len(out_names)
    sm_kw = dict(mesh=mesh, in_specs=(PartitionSpec("core"),) * nio,
                 out_specs=(PartitionSpec("core"),) * len(out_names))
    try:
        smapped = shard_map(_body, check_vma=False, **sm_kw)
    except TypeError:
        smapped = shard_map(_body, check_rep=False, **sm_kw)
    sharded = jax.jit(smapped, donate_argnums=donate, keep_unused=True)
    sh = NamedSharding(mesh, PartitionSpec("core"))
    zeros = jax.jit(
        lambda: tuple(jnp.zeros((n_cores * a.shape[0], *a.shape[1:]), a.dtype)
                      for a in out_avals),
        out_shardings=(sh,) * len(out_avals))()
    concat_in = [np.concatenate([np.asarray(m[k]) for m in in_maps], axis=0)
                 for k in in_names]
    outs = [np.asarray(o) for o in sharded(*concat_in, *zeros)]
    per_core = []
    for c in range(n_cores):
        d = {}
        for name, arr in zip(out_names, outs):
            s0 = arr.shape[0] // n_cores
            d[name] = arr[c * s0:(c + 1) * s0]
        per_core.append(d)
    return per_core


def kernel(x, w_q, w_k, w_v, freqs_cos, freqs_sin, _want_results=False, _trace=False):
    if "nc" not in _CACHE:
        _CACHE["nc"] = _build()
    nc = _CACHE["nc"]
    in_maps = _prep_inputs(np.asarray(x, np.float32), np.asarray(w_q, np.float32),
                           np.asarray(w_k, np.float32), np.asarray(w_v, np.float32),
                           np.asarray(freqs_cos, np.float32),
                           np.asarray(freqs_sin, np.float32))
    if _trace:
        kr = run_bass_kernel_spmd(nc, in_maps, core_ids=list(range(8)), trace=True)
        out = _assemble(kr.results)
        return (out, kr) if _want_results else out
    try:
        results = _run_pjrt(nc, in_maps)
    except Exception as e:
        print(f"kernel: _run_pjrt failed ({type(e).__name__}: {e}); "
              "falling back to run_bass_kernel_spmd", file=sys.stderr)
        kr = run_bass_kernel_spmd(nc, in_maps, core_ids=list(range(8)))
        results = kr.results
    out = _assemble(results)
    if _want_results:
        return out, results
    return out


# revision 37
# speedup vs baseline: 1.0162x; 1.0020x over previous
"""Causal single-head attention (B=2, S=4096, D=1024) + RoPE on 8 TRN2 cores.

Collective-free design: cores 4b+i (b=batch, i=rank 0..3) each receive the
FULL weights and the full x^T of their batch from the host, so no weight or
KV AllGather is needed.  Each core:

  1. projects+ropes Q^T for its own 8 q-chunks QCH[i] (balanced causal mass),
  2. projects K^T directly in transposed layout (stationary=W^T, moving=x^T)
     for ALL 4096 rows and ropes it in transposed layout,
  3. projects V naturally for all rows, interleaved with
  4. 8 attention slots: slot t attends the first 512*(t+1) kv columns; the
     causal edge mask is folded into the scores PSUM accumulation via one
     matmul (stationary=identity, moving=host-built mask), softmax runs
     without max-subtraction (|scores*scale| <= ~3, f32-exact), and exp reads
     PSUM tiles directly with accum_out partial row sums.

Output: final softmax-normalized rows (bf16); host scatters them.
"""

import sys

sys.path.insert(0, "/opt/trn_rl_repo")

import math
from contextlib import ExitStack

import ml_dtypes
import numpy as np

import concourse.bass as bass
import concourse.tile as tile
from concourse import bacc, mybir
from concourse.bass_utils import run_bass_kernel_spmd
from concourse.masks import make_identity

BF16 = mybir.dt.bfloat16
F32 = mybir.dt.float32
NPBF16 = ml_dtypes.bfloat16

B, S, D = 2, 4096, 1024
H = D // 2
C = 128
NQC = S // C                  # 32 chunks of 128 rows
NOC = 8                       # own q chunks per core
NOR = NOC * C                 # 1024 own q rows
NBLK = S // 512               # 8 512-row blocks
SCALE = 1.0 / math.sqrt(D)
NEG = -30000.0

QCH = [sorted([4 * t + i for t in range(4)] + [4 * t + 3 - i for t in range(4, 8)])
       for i in range(4)]

_CACHE = {}


def _emit_qchunk(nc, qstr_p, qraw_p, ktmp_p, qps_p, csq, wq_sb, xq_sb, qt_sb, blk):
    """Project (transposed layout) + rope own q chunk #blk."""
    qcols = slice(blk * C, (blk + 1) * C)
    cq = qstr_p.tile([C, 8, C], BF16, tag="cq", name=f"cq_{blk}")
    nc.sync.dma_start(cq[:], csq[:, :, qcols])
    qraw_c = qraw_p.tile([C, 8, C], BF16, tag="qraw", name=f"qraw_{blk}")
    for ec in range(8):
        qp = qps_p.tile([C, C], F32, tag="qps", name=f"qp_{blk}_{ec}")
        for dc in range(8):
            nc.tensor.matmul(qp[:], wq_sb[:, dc, ec * C:(ec + 1) * C],
                             xq_sb[:, dc, qcols],
                             start=(dc == 0), stop=(dc == 7))
        nc.scalar.copy(qraw_c[:, ec, :], qp[:])
    for pr in range(4):
        cc, ss = cq[:, pr, :], cq[:, pr + 4, :]
        re, im = qraw_c[:, pr, :], qraw_c[:, pr + 4, :]
        t0 = ktmp_p.tile([C, C], BF16, tag="qt0", name=f"qt0_{blk}_{pr}")
        t1 = ktmp_p.tile([C, C], BF16, tag="qt1", name=f"qt1_{blk}_{pr}")
        nc.vector.tensor_mul(t0[:], re, cc)
        nc.vector.tensor_mul(t1[:], im, ss)
        nc.vector.tensor_sub(qt_sb[:, pr, qcols], t0[:], t1[:])
        t2 = ktmp_p.tile([C, C], BF16, tag="qt2", name=f"qt2_{blk}_{pr}")
        t3 = ktmp_p.tile([C, C], BF16, tag="qt3", name=f"qt3_{blk}_{pr}")
        nc.vector.tensor_mul(t2[:], re, ss)
        nc.vector.tensor_mul(t3[:], im, cc)
        nc.vector.tensor_add(qt_sb[:, pr + 4, qcols], t2[:], t3[:])


def _build():
    nc = bacc.Bacc("TRN2", target_bir_lowering=False, debug=False,
                   enable_asserts=False, num_devices=8)

    xt = nc.dram_tensor("xt", [C, 8, S], BF16, kind="ExternalInput").ap()
    xq = nc.dram_tensor("xq", [C, 8, NOR], BF16, kind="ExternalInput").ap()
    csq = nc.dram_tensor("csq", [C, 8, NOR], BF16, kind="ExternalInput").ap()
    csk = nc.dram_tensor("csk", [C, 8, S], BF16, kind="ExternalInput").ap()
    w_in = nc.dram_tensor("w_in", [C, 24, D], BF16, kind="ExternalInput").ap()
    masks = nc.dram_tensor("masks", [C, 2, 512], BF16, kind="ExternalInput").ap()

    o_fin = nc.dram_tensor("o_fin", [NOC, C, D], BF16, kind="ExternalOutput").ap()

    with tile.TileContext(nc) as tc, ExitStack() as ctx:
        const_p = ctx.enter_context(tc.tile_pool(name="const", bufs=1))
        ident = const_p.tile([C, C], BF16)
        make_identity(nc, ident[:])
        masks_sb = const_p.tile([C, 2, 512], BF16)
        nc.sync.dma_start(masks_sb[:], masks)

        # PE warmup: ~11us of dummy transposes while the first weight/x DMAs
        # stream, so the tensor engine is at full p-state when real matmuls
        # start (the results are never read). Source is a fast DVE memset
        # tile so warmup starts before the Pool-built identity is ready.
        with tc.tile_pool(name="wup", bufs=1, space="PSUM") as wup_p, \
             tc.tile_pool(name="wsrc", bufs=1) as wsrc_p:
            wsrc = wsrc_p.tile([C, C], BF16, tag="wsrc")
            nc.vector.memset(wsrc[:], 0.0)
            wup = wup_p.tile([C, C], BF16, tag="wup")
            for r in range(178):
                nc.tensor.transpose(wup[:], wsrc[:], wsrc[:])

        qt_p = ctx.enter_context(tc.tile_pool(name="qt", bufs=1))
        qt_sb = qt_p.tile([C, 8, NOR], BF16, tag="qt")
        # resident until the end: K^T and the V-projection weights (prefetched)
        kt_p = ctx.enter_context(tc.tile_pool(name="kt", bufs=1))
        kt_sb = kt_p.tile([C, 8, S], BF16, tag="kt")
        wv_sb = kt_p.tile([C, 8, D], BF16, tag="wv")

        # ---- phase 1: streamed K^T projection+rope for all rows, with the
        #      own-q chunk of each block projected+roped along the way ----
        with tc.tile_pool(name="ph1", bufs=1) as p1_p, \
             tc.tile_pool(name="kstr", bufs=2) as kstr_p, \
             tc.tile_pool(name="cstr", bufs=2) as cstr_p, \
             tc.tile_pool(name="qstr", bufs=2) as qstr_p, \
             tc.tile_pool(name="kraw", bufs=4) as kraw_p, \
             tc.tile_pool(name="qraw", bufs=2) as qraw_p, \
             tc.tile_pool(name="ktmp", bufs=2) as ktmp_p, \
             tc.tile_pool(name="kps", bufs=4, space="PSUM") as kps_p, \
             tc.tile_pool(name="qps", bufs=3, space="PSUM") as qps_p:
            # DMA issue order = order of first use: K block 0 only needs
            # wk+xb0 (8.7us of DMA) -> PE starts at ~11us; Q-chunk inputs
            # stream while K block 0 runs and fill later block boundaries.
            wk_sb = p1_p.tile([C, 8, D], BF16, tag="wk")
            nc.sync.dma_start(wk_sb[:], w_in[:, 8:16, :])
            xb0 = kstr_p.tile([C, 8, 512], BF16, tag="xb", name="xb_0")
            nc.sync.dma_start(xb0[:], xt[:, :, 0:512])
            cb0 = cstr_p.tile([C, 8, 512], BF16, tag="cb", name="cb_0")
            nc.sync.dma_start(cb0[:], csk[:, :, 0:512])
            wq_sb = p1_p.tile([C, 8, D], BF16, tag="wq")
            nc.sync.dma_start(wq_sb[:], w_in[:, 0:8, :])
            xq_sb = p1_p.tile([C, 8, NOR], BF16, tag="xq")
            nc.sync.dma_start(xq_sb[:, :, 0:2 * C], xq[:, :, 0:2 * C])
            for blk in range(NBLK):
                if blk == 1:
                    nc.sync.dma_start(xq_sb[:, :, 2 * C:NOR], xq[:, :, 2 * C:NOR])
                if blk == 2:
                    # wv needed only in phase V; issue behind the early x blocks
                    nc.sync.dma_start(wv_sb[:], w_in[:, 16:24, :])
                rows = slice(blk * 512, (blk + 1) * 512)
                if blk == 0:
                    xb, cb = xb0, cb0
                else:
                    xb = kstr_p.tile([C, 8, 512], BF16, tag="xb", name=f"xb_{blk}")
                    nc.sync.dma_start(xb[:], xt[:, :, rows])
                    cb = cstr_p.tile([C, 8, 512], BF16, tag="cb", name=f"cb_{blk}")
                    nc.sync.dma_start(cb[:], csk[:, :, rows])
                for pr in range(4):
                    kraw = []
                    for dc in (pr, pr + 4):
                        ps = kps_p.tile([C, 512], F32, tag="kps",
                                        name=f"kps_{blk}_{dc}")
                        for dcd in range(8):
                            nc.tensor.matmul(ps[:],
                                             wk_sb[:, dcd, dc * C:(dc + 1) * C],
                                             xb[:, dcd, :],
                                             start=(dcd == 0), stop=(dcd == 7))
                        kr = kraw_p.tile([C, 512], BF16, tag="kraw",
                                         name=f"kraw_{blk}_{dc}")
                        nc.scalar.copy(kr[:], ps[:])
                        kraw.append(kr)
                    re, im = kraw[0], kraw[1]
                    cc, ss = cb[:, pr, :], cb[:, pr + 4, :]
                    t0 = ktmp_p.tile([C, 512], BF16, tag="kt0", name=f"kt0_{blk}_{pr}")
                    t1 = ktmp_p.tile([C, 512], BF16, tag="kt1", name=f"kt1_{blk}_{pr}")
                    nc.vector.tensor_mul(t0[:], re[:], cc)
                    nc.vector.tensor_mul(t1[:], im[:], ss)
                    nc.vector.tensor_sub(kt_sb[:, pr, rows], t0[:], t1[:])
                    t2 = ktmp_p.tile([C, 512], BF16, tag="kt2", name=f"kt2_{blk}_{pr}")
                    t3 = ktmp_p.tile([C, 512], BF16, tag="kt3", name=f"kt3_{blk}_{pr}")
                    nc.vector.tensor_mul(t2[:], re[:], ss)
                    nc.vector.tensor_mul(t3[:], im[:], cc)
                    nc.vector.tensor_add(kt_sb[:, pr + 4, rows], t2[:], t3[:])

                _emit_qchunk(nc, qstr_p, qraw_p, ktmp_p, qps_p, csq,
                             wq_sb, xq_sb, qt_sb, blk)

        # ---- phase V + attention, interleaved per 512-block/slot ----
        v_p = ctx.enter_context(tc.tile_pool(name="v", bufs=1))
        v_sb = v_p.tile([C, NQC, D], BF16, tag="v")
        with tc.tile_pool(name="vstr", bufs=2) as vstr_p, \
             tc.tile_pool(name="pp", bufs=2) as pp_p, \
             tc.tile_pool(name="at", bufs=2) as at_p, \
             tc.tile_pool(name="ptp", bufs=9) as pt_p, \
             tc.tile_pool(name="vps", bufs=2, space="PSUM") as vps_p, \
             tc.tile_pool(name="scps", bufs=2, space="PSUM") as scps_p, \
             tc.tile_pool(name="ops", bufs=2, space="PSUM") as ops_p, \
             tc.tile_pool(name="ptps", bufs=2, space="PSUM") as ptps_p:
            for t in range(NOC):
                # V projection for kv block t (chunks 4t..4t+3)
                rows = slice(t * 512, (t + 1) * 512)
                xb = vstr_p.tile([C, 8, 512], BF16, tag="vxb", name=f"vxb_{t}")
                nc.sync.dma_start(xb[:], xt[:, :, rows])
                for c4 in range(4):
                    rsl = slice(c4 * C, (c4 + 1) * C)
                    for h in range(2):
                        cols = slice(h * 512, (h + 1) * 512)
                        vp = vps_p.tile([C, 512], F32, tag="vps",
                                        name=f"vps_{t}_{c4}_{h}")
                        for dc in range(8):
                            nc.tensor.matmul(vp[:], xb[:, dc, rsl],
                                             wv_sb[:, dc, cols],
                                             start=(dc == 0), stop=(dc == 7))
                        nc.scalar.copy(v_sb[:, 4 * t + c4, cols], vp[:])

                # attention slot t: q chunk QCH[i][t], kv cols [0, 512*(t+1))
                kpat = 0 if t < 4 else 1
                qc = slice(t * C, (t + 1) * C)
                W = 512 * (t + 1)
                p_sb = pp_p.tile([C, S], BF16, tag="p", name=f"p_{t}")
                lsump = at_p.tile([C, NOC], F32, tag="lsump", name=f"lsump_{t}")
                for u in range(t + 1):
                    cols = slice(u * 512, (u + 1) * 512)
                    sps = scps_p.tile([C, 512], F32, tag="scps", name=f"sps_{t}_{u}")
                    for dc in range(8):
                        nc.tensor.matmul(sps[:], qt_sb[:, dc, qc],
                                         kt_sb[:, dc, cols],
                                         start=(dc == 0),
                                         stop=(dc == 7 and u != t))
                    if u == t:
                        nc.tensor.matmul(sps[:], ident[:], masks_sb[:, kpat, :],
                                         start=False, stop=True)
                    nc.scalar.activation(p_sb[:, cols], sps[:],
                                         mybir.ActivationFunctionType.Exp,
                                         scale=SCALE,
                                         accum_out=lsump[:, u:u + 1])
                lsum = at_p.tile([C, 1], F32, tag="lsum", name=f"lsum_{t}")
                nc.vector.tensor_reduce(lsum[:], lsump[:, 0:t + 1],
                                        axis=mybir.AxisListType.X,
                                        op=mybir.AluOpType.add)
                rinv = at_p.tile([C, 1], F32, tag="rinv", name=f"rinv_{t}")
                nc.vector.reciprocal(rinv[:], lsum[:])

                ob = at_p.tile([C, D], BF16, tag="ob", name=f"ob_{t}")
                pts = []
                for u in range(t + 1):
                    ptp = ptps_p.tile([C, 512], BF16, tag="ptps", name=f"ptp_{t}_{u}")
                    for j in range(4):
                        nc.tensor.transpose(ptp[:, j * C:(j + 1) * C],
                                            p_sb[:, (4 * u + j) * C:(4 * u + j + 1) * C],
                                            ident[:])
                    pt_sb = pt_p.tile([C, 512], BF16, tag="pt", name=f"pt_{t}_{u}")
                    nc.vector.tensor_copy(pt_sb[:], ptp[:])
                    pts.append(pt_sb)
                # final slot: 256-wide output pieces shorten the end-of-kernel
                # scale+store chain behind the last PV accumulation
                nq = 4 if t == NOC - 1 else 2
                qw = D // nq
                for h in range(nq):
                    cols = slice(h * qw, (h + 1) * qw)
                    o_ps = ops_p.tile([C, qw], F32, tag="ops",
                                      name=f"ops_{t}_{h}")
                    for u in range(t + 1):
                        for j in range(4):
                            nc.tensor.matmul(o_ps[:],
                                             pts[u][:, j * C:(j + 1) * C],
                                             v_sb[:, 4 * u + j, cols],
                                             start=(u == 0 and j == 0),
                                             stop=(u == t and j == 3))
                    if h % 2 == 0:
                        nc.vector.tensor_scalar_mul(ob[:, cols], o_ps[:],
                                                    rinv[:])
                    else:
                        nc.scalar.mul(ob[:, cols], o_ps[:], rinv[:])
                    nc.sync.dma_start(o_fin[t, :, cols], ob[:, cols])

    nc.compile()
    return nc


def _xt_blocked(rows_x):
    """[n, D] float -> [C, 8, n] bf16 blocked transpose."""
    return np.ascontiguousarray(
        rows_x.astype(NPBF16).reshape(-1, 8, C).transpose(2, 1, 0))


def _masks(i):
    tri = np.where(np.arange(C)[:, None] >= np.arange(C)[None, :], 0.0, NEG)
    m = np.zeros((C, 2, 512), np.float32)
    for k, diag in enumerate((i, 3 - i)):
        for c in range(4):
            if c > diag:
                m[:, k, c * C:(c + 1) * C] = NEG
            elif c == diag:
                m[:, k, c * C:(c + 1) * C] = tri
    return np.ascontiguousarray(m.astype(NPBF16))


def _prep_inputs(x, w_q, w_k, w_v, freqs_cos, freqs_sin):
    perm = np.concatenate([np.arange(0, D, 2), np.arange(1, D, 2)])
    wqT = np.ascontiguousarray(w_q[perm, :].T)
    wkT = np.ascontiguousarray(w_k[perm, :].T)
    wvT = np.ascontiguousarray(w_v.T)

    def blk(wt):  # [D, D] -> [C, dc, e]
        return wt.astype(NPBF16).reshape(8, C, D).transpose(1, 0, 2)

    flat24 = np.ascontiguousarray(
        np.concatenate([blk(wqT), blk(wkT), blk(wvT)], axis=1))  # [C, 24, D]

    cs_all = np.concatenate([freqs_cos, freqs_sin], axis=1)  # [S, D]
    csk_b = _xt_blocked(cs_all)                              # same for all cores
    xt_b = [_xt_blocked(np.asarray(x[b])) for b in range(B)]

    in_maps = []
    for core in range(8):
        b, i = divmod(core, 4)
        qrows = (np.asarray(QCH[i])[:, None] * C + np.arange(C)[None, :]).reshape(-1)
        in_maps.append({
            "xt": xt_b[b],
            "xq": _xt_blocked(np.asarray(x[b])[qrows]),
            "csq": _xt_blocked(cs_all[qrows]),
            "csk": csk_b,
            "w_in": flat24,
            "masks": _masks(i),
        })
    return in_maps


def _assemble(results):
    out = np.empty((B, S, D), np.float32)
    for core in range(8):
        b, i = divmod(core, 4)
        o = np.asarray(results[core]["o_fin"], np.float32)  # [NOC, C, D]
        for t, j in enumerate(QCH[i]):
            out[b, j * C:(j + 1) * C] = o[t]
    return out


def _run_pjrt(nc, in_maps, n_cores=8):
    """Like bass2jax.run_bass_via_pjrt, but creates the donated output
    buffers ON DEVICE (jit zeros) instead of uploading host zeros."""
    import jax
    import jax.numpy as jnp
    from jax.sharding import Mesh, NamedSharding, PartitionSpec
    try:
        from jax import shard_map
    except ImportError:
        from jax.experimental.shard_map import shard_map
    from concourse.bass2jax import (_bass_exec_p, install_neuronx_cc_hook,
                                    partition_id_tensor)

    install_neuronx_cc_hook()
    partition_name = nc.partition_id_tensor.name if nc.partition_id_tensor else None
    in_names, out_names, out_avals = [], [], []
    for alloc in nc.m.functions[0].allocations:
        if not isinstance(alloc, mybir.MemoryLocationSet):
            continue
        name = alloc.memorylocations[0].name
        if alloc.kind == "ExternalInput":
            if name != partition_name:
                in_names.append(name)
        elif alloc.kind == "ExternalOutput":
            out_avals.append(jax.core.ShapedArray(
                tuple(alloc.tensor_shape), mybir.dt.np(alloc.dtype)))
            out_names.append(name)
    n_params = len(in_names)
    all_in = list(in_names) + list(out_names)
    if partition_name is not None:
        all_in.append(partition_name)
    donate = tuple(range(n_params, n_params + len(out_names)))

    def _body(*args):
        operands = list(args)
        if partition_name is not None:
            operands.append(partition_id_tensor())
        return tuple(_bass_exec_p.bind(
            *operands, out_avals=tuple(out_avals), in_names=tuple(all_in),
            out_names=tuple(out_names), lowering_input_output_aliases=(),
            sim_require_finite=True, sim_require_nnan=True, nc=nc))

    devices = jax.devices()[:n_cores]
    mesh = Mesh(np.asarray(devices), ("core",))
    nio = n_params + len(out_names)
    sm_kw = dict(mesh=mesh, in_specs=(PartitionSpec("core"),) * nio,
                 out_specs=(PartitionSpec("core"),) * len(out_names))
    try:
        smapped = shard_map(_body, check_vma=False, **sm_kw)
    except TypeError:
        smapped = shard_map(_body, check_rep=False, **sm_kw)
    sharded = jax.jit(smapped, donate_argnums=donate, keep_unused=True)
    sh = NamedSharding(mesh, PartitionSpec("core"))
    zeros = jax.jit(
        lambda: tuple(jnp.zeros((n_cores * a.shape[0], *a.shape[1:]), a.dtype)
                      for a in out_avals),
        out_shardings=(sh,) * len(out_avals))()
    concat_in = [np.concatenate([np.asarray(m[k]) for m in in_maps], axis=0)
                 for k in in_names]
    outs = [np.asarray(o) for o in sharded(*concat_in, *zeros)]
    per_core = []
    for c in range(n_cores):
        d = {}
        for name, arr in zip(out_names, outs):
            s0 = arr.shape[0] // n_cores
            d[name] = arr[c * s0:(c + 1) * s0]
        per_core.append(d)
    return per_core


def kernel(x, w_q, w_k, w_v, freqs_cos, freqs_sin, _want_results=False, _trace=False):
    if "nc" not in _CACHE:
        _CACHE["nc"] = _build()
    nc = _CACHE["nc"]
    in_maps = _prep_inputs(np.asarray(x, np.float32), np.asarray(w_q, np.float32),
                           np.asarray(w_k, np.float32), np.asarray(w_v, np.float32),
                           np.asarray(freqs_cos, np.float32),
                           np.asarray(freqs_sin, np.float32))
    if _trace:
        kr = run_bass_kernel_spmd(nc, in_maps, core_ids=list(range(8)), trace=True)
        out = _assemble(kr.results)
        return (out, kr) if _want_results else out
    try:
        results = _run_pjrt(nc, in_maps)
    except Exception as e:
        print(f"kernel: _run_pjrt failed ({type(e).__name__}: {e}); "
              "falling back to run_bass_kernel_spmd", file=sys.stderr)
        kr = run_bass_kernel_spmd(nc, in_maps, core_ids=list(range(8)))
        results = kr.results
    out = _assemble(results)
    if _want_results:
        return out, results
    return out


# revision 44
# speedup vs baseline: 1.0175x; 1.0013x over previous
"""Causal single-head attention (B=2, S=4096, D=1024) + RoPE on 8 TRN2 cores.

Collective-free design: cores 4b+i (b=batch, i=rank 0..3) each receive the
FULL weights and the full x^T of their batch from the host, so no weight or
KV AllGather is needed.  Each core:

  1. projects+ropes Q^T for its own 8 q-chunks QCH[i] (balanced causal mass),
  2. projects K^T directly in transposed layout (stationary=W^T, moving=x^T)
     for ALL 4096 rows and ropes it in transposed layout,
  3. projects V naturally for all rows, interleaved with
  4. 8 attention slots: slot t attends the first 512*(t+1) kv columns; the
     causal edge mask is folded into the scores PSUM accumulation via one
     matmul (stationary=identity, moving=host-built mask), softmax runs
     without max-subtraction (|scores*scale| <= ~3, f32-exact), and exp reads
     PSUM tiles directly with accum_out partial row sums.

Output: final softmax-normalized rows (bf16); host scatters them.
"""

import sys

sys.path.insert(0, "/opt/trn_rl_repo")

import math
from contextlib import ExitStack

import ml_dtypes
import numpy as np

import concourse.bass as bass
import concourse.tile as tile
from concourse import bacc, mybir
from concourse.bass_utils import run_bass_kernel_spmd
from concourse.masks import make_identity

BF16 = mybir.dt.bfloat16
F32 = mybir.dt.float32
NPBF16 = ml_dtypes.bfloat16

B, S, D = 2, 4096, 1024
H = D // 2
C = 128
NQC = S // C                  # 32 chunks of 128 rows
NOC = 8                       # own q chunks per core
NOR = NOC * C                 # 1024 own q rows
NBLK = S // 512               # 8 512-row blocks
SCALE = 1.0 / math.sqrt(D)
NEG = -30000.0

QCH = [sorted([4 * t + i for t in range(4)] + [4 * t + 3 - i for t in range(4, 8)])
       for i in range(4)]

_CACHE = {}


def _emit_qchunk(nc, qstr_p, qraw_p, ktmp_p, qps_p, csq, wq_sb, xq_sb, qt_sb, blk):
    """Project (transposed layout) + rope own q chunk #blk."""
    qcols = slice(blk * C, (blk + 1) * C)
    cq = qstr_p.tile([C, 8, C], BF16, tag="cq", name=f"cq_{blk}")
    nc.sync.dma_start(cq[:], csq[:, :, qcols])
    qraw_c = qraw_p.tile([C, 8, C], BF16, tag="qraw", name=f"qraw_{blk}")
    for ec in range(8):
        qp = qps_p.tile([C, C], F32, tag="qps", name=f"qp_{blk}_{ec}")
        for dc in range(8):
            nc.tensor.matmul(qp[:], wq_sb[:, dc, ec * C:(ec + 1) * C],
                             xq_sb[:, dc, qcols],
                             start=(dc == 0), stop=(dc == 7))
        nc.scalar.copy(qraw_c[:, ec, :], qp[:])
    for pr in range(4):
        cc, ss = cq[:, pr, :], cq[:, pr + 4, :]
        re, im = qraw_c[:, pr, :], qraw_c[:, pr + 4, :]
        t0 = ktmp_p.tile([C, C], BF16, tag="qt0", name=f"qt0_{blk}_{pr}")
        t1 = ktmp_p.tile([C, C], BF16, tag="qt1", name=f"qt1_{blk}_{pr}")
        nc.vector.tensor_mul(t0[:], re, cc)
        nc.vector.tensor_mul(t1[:], im, ss)
        nc.vector.tensor_sub(qt_sb[:, pr, qcols], t0[:], t1[:])
        t2 = ktmp_p.tile([C, C], BF16, tag="qt2", name=f"qt2_{blk}_{pr}")
        t3 = ktmp_p.tile([C, C], BF16, tag="qt3", name=f"qt3_{blk}_{pr}")
        nc.vector.tensor_mul(t2[:], re, ss)
        nc.vector.tensor_mul(t3[:], im, cc)
        nc.vector.tensor_add(qt_sb[:, pr + 4, qcols], t2[:], t3[:])


def _build():
    nc = bacc.Bacc("TRN2", target_bir_lowering=False, debug=False,
                   enable_asserts=False, num_devices=8)

    xt = nc.dram_tensor("xt", [C, 8, S], BF16, kind="ExternalInput").ap()
    xq = nc.dram_tensor("xq", [C, 8, NOR], BF16, kind="ExternalInput").ap()
    csq = nc.dram_tensor("csq", [C, 8, NOR], BF16, kind="ExternalInput").ap()
    csk = nc.dram_tensor("csk", [C, 8, S], BF16, kind="ExternalInput").ap()
    w_in = nc.dram_tensor("w_in", [C, 24, D], BF16, kind="ExternalInput").ap()
    masks = nc.dram_tensor("masks", [C, 2, 512], BF16, kind="ExternalInput").ap()

    o_fin = nc.dram_tensor("o_fin", [NOC, C, D], BF16, kind="ExternalOutput").ap()

    with tile.TileContext(nc) as tc, ExitStack() as ctx:
        const_p = ctx.enter_context(tc.tile_pool(name="const", bufs=1))
        ident = const_p.tile([C, C], BF16)
        make_identity(nc, ident[:])
        masks_sb = const_p.tile([C, 2, 512], BF16)

        # PE warmup: ~11us of dummy transposes while the first weight/x DMAs
        # stream, so the tensor engine is at full p-state when real matmuls
        # start (the results are never read). Source is a fast DVE memset
        # tile so warmup starts before the Pool-built identity is ready.
        with tc.tile_pool(name="wup", bufs=1, space="PSUM") as wup_p, \
             tc.tile_pool(name="wsrc", bufs=1) as wsrc_p:
            wsrc = wsrc_p.tile([C, C], BF16, tag="wsrc")
            nc.vector.memset(wsrc[:], 0.0)
            wup = wup_p.tile([C, C], BF16, tag="wup")
            for r in range(167):
                nc.tensor.transpose(wup[:], wsrc[:], wsrc[:])

        qt_p = ctx.enter_context(tc.tile_pool(name="qt", bufs=1))
        qt_sb = qt_p.tile([C, 8, NOR], BF16, tag="qt")
        # resident until the end: K^T and the V-projection weights (prefetched)
        kt_p = ctx.enter_context(tc.tile_pool(name="kt", bufs=1))
        kt_sb = kt_p.tile([C, 8, S], BF16, tag="kt")
        wv_sb = kt_p.tile([C, 8, D], BF16, tag="wv")

        # ---- phase 1: streamed K^T projection+rope for all rows, with the
        #      own-q chunk of each block projected+roped along the way ----
        with tc.tile_pool(name="ph1", bufs=1) as p1_p, \
             tc.tile_pool(name="kstr", bufs=2) as kstr_p, \
             tc.tile_pool(name="cstr", bufs=2) as cstr_p, \
             tc.tile_pool(name="qstr", bufs=2) as qstr_p, \
             tc.tile_pool(name="kraw", bufs=4) as kraw_p, \
             tc.tile_pool(name="qraw", bufs=2) as qraw_p, \
             tc.tile_pool(name="ktmp", bufs=2) as ktmp_p, \
             tc.tile_pool(name="kps", bufs=4, space="PSUM") as kps_p, \
             tc.tile_pool(name="qps", bufs=3, space="PSUM") as qps_p:
            # DMA issue order = order of first use: K block 0 only needs
            # wk+xb0 (8.7us of DMA) -> PE starts at ~11us; Q-chunk inputs
            # stream while K block 0 runs and fill later block boundaries.
            wk_sb = p1_p.tile([C, 8, D], BF16, tag="wk")
            nc.sync.dma_start(wk_sb[:], w_in[:, 8:16, :])
            xb0 = kstr_p.tile([C, 8, 512], BF16, tag="xb", name="xb_0")
            nc.sync.dma_start(xb0[:], xt[:, :, 0:512])
            cb0 = cstr_p.tile([C, 8, 512], BF16, tag="cb", name="cb_0")
            nc.sync.dma_start(cb0[:], csk[:, :, 0:512])
            wq_sb = p1_p.tile([C, 8, D], BF16, tag="wq")
            nc.sync.dma_start(wq_sb[:], w_in[:, 0:8, :])
            xq_sb = p1_p.tile([C, 8, NOR], BF16, tag="xq")
            nc.sync.dma_start(xq_sb[:, :, 0:2 * C], xq[:, :, 0:2 * C])
            for blk in range(NBLK):
                if blk == 1:
                    nc.sync.dma_start(xq_sb[:, :, 2 * C:NOR], xq[:, :, 2 * C:NOR])
                if blk == 2:
                    # wv/masks needed only in phase V; issue behind the early
                    # x blocks
                    nc.sync.dma_start(wv_sb[:], w_in[:, 16:24, :])
                    nc.sync.dma_start(masks_sb[:], masks)
                rows = slice(blk * 512, (blk + 1) * 512)
                if blk == 0:
                    xb, cb = xb0, cb0
                else:
                    xb = kstr_p.tile([C, 8, 512], BF16, tag="xb", name=f"xb_{blk}")
                    nc.sync.dma_start(xb[:], xt[:, :, rows])
                    cb = cstr_p.tile([C, 8, 512], BF16, tag="cb", name=f"cb_{blk}")
                    nc.sync.dma_start(cb[:], csk[:, :, rows])
                for pr in range(4):
                    kraw = []
                    for dc in (pr, pr + 4):
                        ps = kps_p.tile([C, 512], F32, tag="kps",
                                        name=f"kps_{blk}_{dc}")
                        for dcd in range(8):
                            nc.tensor.matmul(ps[:],
                                             wk_sb[:, dcd, dc * C:(dc + 1) * C],
                                             xb[:, dcd, :],
                                             start=(dcd == 0), stop=(dcd == 7))
                        kr = kraw_p.tile([C, 512], BF16, tag="kraw",
                                         name=f"kraw_{blk}_{dc}")
                        nc.scalar.copy(kr[:], ps[:])
                        kraw.append(kr)
                    re, im = kraw[0], kraw[1]
                    cc, ss = cb[:, pr, :], cb[:, pr + 4, :]
                    t0 = ktmp_p.tile([C, 512], BF16, tag="kt0", name=f"kt0_{blk}_{pr}")
                    t1 = ktmp_p.tile([C, 512], BF16, tag="kt1", name=f"kt1_{blk}_{pr}")
                    nc.vector.tensor_mul(t0[:], re[:], cc)
                    nc.vector.tensor_mul(t1[:], im[:], ss)
                    nc.vector.tensor_sub(kt_sb[:, pr, rows], t0[:], t1[:])
                    t2 = ktmp_p.tile([C, 512], BF16, tag="kt2", name=f"kt2_{blk}_{pr}")
                    t3 = ktmp_p.tile([C, 512], BF16, tag="kt3", name=f"kt3_{blk}_{pr}")
                    nc.vector.tensor_mul(t2[:], re[:], ss)
                    nc.vector.tensor_mul(t3[:], im[:], cc)
                    nc.vector.tensor_add(kt_sb[:, pr + 4, rows], t2[:], t3[:])

                _emit_qchunk(nc, qstr_p, qraw_p, ktmp_p, qps_p, csq,
                             wq_sb, xq_sb, qt_sb, blk)

        # ---- phase V + attention, interleaved per 512-block/slot ----
        v_p = ctx.enter_context(tc.tile_pool(name="v", bufs=1))
        v_sb = v_p.tile([C, NQC, D], BF16, tag="v")
        with tc.tile_pool(name="vstr", bufs=2) as vstr_p, \
             tc.tile_pool(name="pp", bufs=2) as pp_p, \
             tc.tile_pool(name="at", bufs=2) as at_p, \
             tc.tile_pool(name="ptp", bufs=9) as pt_p, \
             tc.tile_pool(name="vps", bufs=2, space="PSUM") as vps_p, \
             tc.tile_pool(name="scps", bufs=2, space="PSUM") as scps_p, \
             tc.tile_pool(name="ops", bufs=2, space="PSUM") as ops_p, \
             tc.tile_pool(name="ptps", bufs=2, space="PSUM") as ptps_p:

            def _emit_slot(t):
                kpat = 0 if t < 4 else 1
                qc = slice(t * C, (t + 1) * C)
                p_sb = pp_p.tile([C, S], BF16, tag="p", name=f"p_{t}")
                lsump = at_p.tile([C, NOC], F32, tag="lsump", name=f"lsump_{t}")
                for u in range(t + 1):
                    cols = slice(u * 512, (u + 1) * 512)
                    sps = scps_p.tile([C, 512], F32, tag="scps",
                                      name=f"sps_{t}_{u}")
                    for dc in range(8):
                        nc.tensor.matmul(sps[:], qt_sb[:, dc, qc],
                                         kt_sb[:, dc, cols],
                                         start=(dc == 0),
                                         stop=(dc == 7 and u != t))
                    if u == t:
                        nc.tensor.matmul(sps[:], ident[:], masks_sb[:, kpat, :],
                                         start=False, stop=True)
                    nc.scalar.activation(p_sb[:, cols], sps[:],
                                         mybir.ActivationFunctionType.Exp,
                                         scale=SCALE,
                                         accum_out=lsump[:, u:u + 1])
                lsum = at_p.tile([C, 1], F32, tag="lsum", name=f"lsum_{t}")
                nc.vector.tensor_reduce(lsum[:], lsump[:, 0:t + 1],
                                        axis=mybir.AxisListType.X,
                                        op=mybir.AluOpType.add)
                rinv = at_p.tile([C, 1], F32, tag="rinv", name=f"rinv_{t}")
                nc.vector.reciprocal(rinv[:], lsum[:])

                ob = at_p.tile([C, D], BF16, tag="ob", name=f"ob_{t}")
                pts = []
                for u in range(t + 1):
                    ptp = ptps_p.tile([C, 512], BF16, tag="ptps",
                                      name=f"ptp_{t}_{u}")
                    for j in range(4):
                        nc.tensor.transpose(
                            ptp[:, j * C:(j + 1) * C],
                            p_sb[:, (4 * u + j) * C:(4 * u + j + 1) * C],
                            ident[:])
                    pt_sb = pt_p.tile([C, 512], BF16, tag="pt", name=f"pt_{t}_{u}")
                    nc.vector.tensor_copy(pt_sb[:], ptp[:])
                    pts.append(pt_sb)
                # final slot: 256-wide output pieces shorten the
                # end-of-kernel scale+store chain
                nq = 4 if t == NOC - 1 else 2
                qw = D // nq
                for h in range(nq):
                    cols = slice(h * qw, (h + 1) * qw)
                    o_ps = ops_p.tile([C, qw], F32, tag="ops",
                                      name=f"ops_{t}_{h}")
                    for u in range(t + 1):
                        for j in range(4):
                            nc.tensor.matmul(o_ps[:],
                                             pts[u][:, j * C:(j + 1) * C],
                                             v_sb[:, 4 * u + j, cols],
                                             start=(u == 0 and j == 0),
                                             stop=(u == t and j == 3))
                    if h % 2 == 0:
                        nc.vector.tensor_scalar_mul(ob[:, cols], o_ps[:],
                                                    rinv[:])
                    else:
                        nc.scalar.mul(ob[:, cols], o_ps[:], rinv[:])
                    nc.sync.dma_start(o_fin[t, :, cols], ob[:, cols])

            for t in range(NOC):
                # V projection for kv block t (chunks 4t..4t+3)
                rows = slice(t * 512, (t + 1) * 512)
                xb = vstr_p.tile([C, 8, 512], BF16, tag="vxb", name=f"vxb_{t}")
                nc.sync.dma_start(xb[:], xt[:, :, rows])
                for c4 in range(4):
                    rsl = slice(c4 * C, (c4 + 1) * C)
                    for h in range(2):
                        cols = slice(h * 512, (h + 1) * 512)
                        vp = vps_p.tile([C, 512], F32, tag="vps",
                                        name=f"vps_{t}_{c4}_{h}")
                        for dc in range(8):
                            nc.tensor.matmul(vp[:], xb[:, dc, rsl],
                                             wv_sb[:, dc, cols],
                                             start=(dc == 0), stop=(dc == 7))
                        nc.scalar.copy(v_sb[:, 4 * t + c4, cols], vp[:])

                # attention slot t: q chunk QCH[i][t], kv cols [0, 512*(t+1))
                _emit_slot(t)

    nc.compile()
    return nc


def _xt_blocked(rows_x):
    """[n, D] float -> [C, 8, n] bf16 blocked transpose."""
    return np.ascontiguousarray(
        rows_x.astype(NPBF16).reshape(-1, 8, C).transpose(2, 1, 0))


def _masks(i):
    tri = np.where(np.arange(C)[:, None] >= np.arange(C)[None, :], 0.0, NEG)
    m = np.zeros((C, 2, 512), np.float32)
    for k, diag in enumerate((i, 3 - i)):
        for c in range(4):
            if c > diag:
                m[:, k, c * C:(c + 1) * C] = NEG
            elif c == diag:
                m[:, k, c * C:(c + 1) * C] = tri
    return np.ascontiguousarray(m.astype(NPBF16))


def _prep_inputs(x, w_q, w_k, w_v, freqs_cos, freqs_sin):
    perm = np.concatenate([np.arange(0, D, 2), np.arange(1, D, 2)])
    wqT = np.ascontiguousarray(w_q[perm, :].T)
    wkT = np.ascontiguousarray(w_k[perm, :].T)
    wvT = np.ascontiguousarray(w_v.T)

    def blk(wt):  # [D, D] -> [C, dc, e]
        return wt.astype(NPBF16).reshape(8, C, D).transpose(1, 0, 2)

    flat24 = np.ascontiguousarray(
        np.concatenate([blk(wqT), blk(wkT), blk(wvT)], axis=1))  # [C, 24, D]

    cs_all = np.concatenate([freqs_cos, freqs_sin], axis=1)  # [S, D]
    csk_b = _xt_blocked(cs_all)                              # same for all cores
    xt_b = [_xt_blocked(np.asarray(x[b])) for b in range(B)]

    in_maps = []
    for core in range(8):
        b, i = divmod(core, 4)
        qrows = (np.asarray(QCH[i])[:, None] * C + np.arange(C)[None, :]).reshape(-1)
        in_maps.append({
            "xt": xt_b[b],
            "xq": _xt_blocked(np.asarray(x[b])[qrows]),
            "csq": _xt_blocked(cs_all[qrows]),
            "csk": csk_b,
            "w_in": flat24,
            "masks": _masks(i),
        })
    return in_maps


def _assemble(results):
    out = np.empty((B, S, D), np.float32)
    for core in range(8):
        b, i = divmod(core, 4)
        o = np.asarray(results[core]["o_fin"], np.float32)  # [NOC, C, D]
        for t, j in enumerate(QCH[i]):
            out[b, j * C:(j + 1) * C] = o[t]
    return out


def _run_pjrt(nc, in_maps, n_cores=8):
    """Like bass2jax.run_bass_via_pjrt, but creates the donated output
    buffers ON DEVICE (jit zeros) instead of uploading host zeros."""
    import jax
    import jax.numpy as jnp
    from jax.sharding import Mesh, NamedSharding, PartitionSpec
    try:
        from jax import shard_map
    except ImportError:
        from jax.experimental.shard_map import shard_map
    from concourse.bass2jax import (_bass_exec_p, install_neuronx_cc_hook,
                                    partition_id_tensor)

    install_neuronx_cc_hook()
    partition_name = nc.partition_id_tensor.name if nc.partition_id_tensor else None
    in_names, out_names, out_avals = [], [], []
    for alloc in nc.m.functions[0].allocations:
        if not isinstance(alloc, mybir.MemoryLocationSet):
            continue
        name = alloc.memorylocations[0].name
        if alloc.kind == "ExternalInput":
            if name != partition_name:
                in_names.append(name)
        elif alloc.kind == "ExternalOutput":
            out_avals.append(jax.core.ShapedArray(
                tuple(alloc.tensor_shape), mybir.dt.np(alloc.dtype)))
            out_names.append(name)
    n_params = len(in_names)
    all_in = list(in_names) + list(out_names)
    if partition_name is not None:
        all_in.append(partition_name)
    donate = tuple(range(n_params, n_params + len(out_names)))

    def _body(*args):
        operands = list(args)
        if partition_name is not None:
            operands.append(partition_id_tensor())
        return tuple(_bass_exec_p.bind(
            *operands, out_avals=tuple(out_avals), in_names=tuple(all_in),
            out_names=tuple(out_names), lowering_input_output_aliases=(),
            sim_require_finite=True, sim_require_nnan=True, nc=nc))

    devices = jax.devices()[:n_cores]
    mesh = Mesh(np.asarray(devices), ("core",))
    nio = n_params + len(out_names)
    sm_kw = dict(mesh=mesh, in_specs=(PartitionSpec("core"),) * nio,
                 out_specs=(PartitionSpec("core"),) * len(out_names))
    try:
        smapped = shard_map(_body, check_vma=False, **sm_kw)
    except TypeError:
        smapped = shard_map(_body, check_rep=False, **sm_kw)
    sharded = jax.jit(smapped, donate_argnums=donate, keep_unused=True)
    sh = NamedSharding(mesh, PartitionSpec("core"))
    zeros = jax.jit(
        lambda: tuple(jnp.zeros((n_cores * a.shape[0], *a.shape[1:]), a.dtype)
                      for a in out_avals),
        out_shardings=(sh,) * len(out_avals))()
    concat_in = [np.concatenate([np.asarray(m[k]) for m in in_maps], axis=0)
                 for k in in_names]
    outs = [np.asarray(o) for o in sharded(*concat_in, *zeros)]
    per_core = []
    for c in range(n_cores):
        d = {}
        for name, arr in zip(out_names, outs):
            s0 = arr.shape[0] // n_cores
            d[name] = arr[c * s0:(c + 1) * s0]
        per_core.append(d)
    return per_core


def kernel(x, w_q, w_k, w_v, freqs_cos, freqs_sin, _want_results=False, _trace=False):
    if "nc" not in _CACHE:
        _CACHE["nc"] = _build()
    nc = _CACHE["nc"]
    in_maps = _prep_inputs(np.asarray(x, np.float32), np.asarray(w_q, np.float32),
                           np.asarray(w_k, np.float32), np.asarray(w_v, np.float32),
                           np.asarray(freqs_cos, np.float32),
                           np.asarray(freqs_sin, np.float32))
    if _trace:
        kr = run_bass_kernel_spmd(nc, in_maps, core_ids=list(range(8)), trace=True)
        out = _assemble(kr.results)
        return (out, kr) if _want_results else out
    try:
        results = _run_pjrt(nc, in_maps)
    except Exception as e:
        print(f"kernel: _run_pjrt failed ({type(e).__name__}: {e}); "
              "falling back to run_bass_kernel_spmd", file=sys.stderr)
        kr = run_bass_kernel_spmd(nc, in_maps, core_ids=list(range(8)))
        results = kr.results
    out = _assemble(results)
    if _want_results:
        return out, results
    return out


# revision 51
# speedup vs baseline: 1.0392x; 1.0213x over previous
"""Causal single-head attention (B=2, S=4096, D=1024) + RoPE on 8 TRN2 cores.

Collective-free design: cores 4b+i (b=batch, i=rank 0..3) each receive the
FULL weights and the full x^T of their batch from the host, so no weight or
KV AllGather is needed.  Each core:

  1. projects+ropes Q^T for its own 8 q-chunks QCH[i] (balanced causal mass),
  2. projects K^T directly in transposed layout (stationary=W^T, moving=x^T)
     for ALL 4096 rows and ropes it in transposed layout,
  3. projects V naturally for all rows, interleaved with
  4. 8 attention slots: slot t attends the first 512*(t+1) kv columns; the
     causal edge mask is folded into the scores PSUM accumulation via one
     matmul (stationary=identity, moving=host-built mask), softmax runs
     without max-subtraction (|scores*scale| <= ~3, f32-exact), and exp reads
     PSUM tiles directly with accum_out partial row sums.

Output: final softmax-normalized rows (bf16); host scatters them.
"""

import sys

sys.path.insert(0, "/opt/trn_rl_repo")

import math
from contextlib import ExitStack

import ml_dtypes
import numpy as np

import concourse.bass as bass
import concourse.tile as tile
from concourse import bacc, mybir
from concourse.bass_utils import run_bass_kernel_spmd
from concourse.masks import make_identity

BF16 = mybir.dt.bfloat16
F32 = mybir.dt.float32
NPBF16 = ml_dtypes.bfloat16

B, S, D = 2, 4096, 1024
H = D // 2
C = 128
NQC = S // C                  # 32 chunks of 128 rows
NOC = 8                       # own q chunks per core
NOR = NOC * C                 # 1024 own q rows
NBLK = S // 512               # 8 512-row blocks
SCALE = 1.0 / math.sqrt(D)
NEG = -30000.0

QCH = [sorted([4 * t + i for t in range(4)] + [4 * t + 3 - i for t in range(4, 8)])
       for i in range(4)]

_CACHE = {}


def _emit_qchunk(nc, qstr_p, qraw_p, ktmp_p, qps_p, csq, wq_sb, xq_sb, qt_sb, blk):
    """Project (transposed layout) + rope own q chunk #blk."""
    qcols = slice(blk * C, (blk + 1) * C)
    cq = qstr_p.tile([C, 8, C], BF16, tag="cq", name=f"cq_{blk}")
    nc.sync.dma_start(cq[:], csq[:, :, qcols])
    qraw_c = qraw_p.tile([C, 8, C], BF16, tag="qraw", name=f"qraw_{blk}")
    for ec in range(8):
        qp = qps_p.tile([C, C], F32, tag="qps", name=f"qp_{blk}_{ec}")
        for dc in range(8):
            nc.tensor.matmul(qp[:], wq_sb[:, dc, ec * C:(ec + 1) * C],
                             xq_sb[:, dc, qcols],
                             start=(dc == 0), stop=(dc == 7))
        nc.scalar.copy(qraw_c[:, ec, :], qp[:])
    for pr in range(4):
        cc, ss = cq[:, pr, :], cq[:, pr + 4, :]
        re, im = qraw_c[:, pr, :], qraw_c[:, pr + 4, :]
        t0 = ktmp_p.tile([C, C], BF16, tag="qt0", name=f"qt0_{blk}_{pr}")
        t1 = ktmp_p.tile([C, C], BF16, tag="qt1", name=f"qt1_{blk}_{pr}")
        nc.vector.tensor_mul(t0[:], re, cc)
        nc.vector.tensor_mul(t1[:], im, ss)
        nc.vector.tensor_sub(qt_sb[:, pr, qcols], t0[:], t1[:])
        t2 = ktmp_p.tile([C, C], BF16, tag="qt2", name=f"qt2_{blk}_{pr}")
        t3 = ktmp_p.tile([C, C], BF16, tag="qt3", name=f"qt3_{blk}_{pr}")
        nc.vector.tensor_mul(t2[:], re, ss)
        nc.vector.tensor_mul(t3[:], im, cc)
        nc.vector.tensor_add(qt_sb[:, pr + 4, qcols], t2[:], t3[:])


def _build():
    nc = bacc.Bacc("TRN2", target_bir_lowering=False, debug=False,
                   enable_asserts=False, num_devices=8)

    xt = nc.dram_tensor("xt", [C, 8, S], BF16, kind="ExternalInput").ap()
    xq = nc.dram_tensor("xq", [C, 8, NOR], BF16, kind="ExternalInput").ap()
    csq = nc.dram_tensor("csq", [C, 8, NOR], BF16, kind="ExternalInput").ap()
    csk = nc.dram_tensor("csk", [C, 8, S], BF16, kind="ExternalInput").ap()
    w_in = nc.dram_tensor("w_in", [C, 24, D], BF16, kind="ExternalInput").ap()
    masks = nc.dram_tensor("masks", [C, 2, 512], BF16, kind="ExternalInput").ap()

    o_fin = nc.dram_tensor("o_fin", [NOC, C, D], BF16, kind="ExternalOutput").ap()

    with tile.TileContext(nc) as tc, ExitStack() as ctx:
        const_p = ctx.enter_context(tc.tile_pool(name="const", bufs=1))
        ident = const_p.tile([C, C], BF16)
        make_identity(nc, ident[:])
        masks_sb = const_p.tile([C, 2, 512], BF16)
        ones_sb = const_p.tile([C, 1], BF16)
        nc.vector.memset(ones_sb[:], 1.0)

        # PE warmup: ~11us of dummy transposes while the first weight/x DMAs
        # stream, so the tensor engine is at full p-state when real matmuls
        # start (the results are never read). Source is a fast DVE memset
        # tile so warmup starts before the Pool-built identity is ready.
        with tc.tile_pool(name="wup", bufs=1, space="PSUM") as wup_p, \
             tc.tile_pool(name="wsrc", bufs=1) as wsrc_p:
            wsrc = wsrc_p.tile([C, C], BF16, tag="wsrc")
            nc.vector.memset(wsrc[:], 0.0)
            wup = wup_p.tile([C, C], BF16, tag="wup")
            for r in range(167):
                nc.tensor.transpose(wup[:], wsrc[:], wsrc[:])

        qt_p = ctx.enter_context(tc.tile_pool(name="qt", bufs=1))
        qt_sb = qt_p.tile([C, 8, NOR], BF16, tag="qt")
        # resident until the end: K^T and the V-projection weights (prefetched)
        kt_p = ctx.enter_context(tc.tile_pool(name="kt", bufs=1))
        kt_sb = kt_p.tile([C, 8, S], BF16, tag="kt")
        wv_sb = kt_p.tile([C, 8, D], BF16, tag="wv")

        # ---- phase 1: streamed K^T projection+rope for all rows, with the
        #      own-q chunk of each block projected+roped along the way ----
        with tc.tile_pool(name="ph1", bufs=1) as p1_p, \
             tc.tile_pool(name="kstr", bufs=2) as kstr_p, \
             tc.tile_pool(name="cstr", bufs=2) as cstr_p, \
             tc.tile_pool(name="qstr", bufs=2) as qstr_p, \
             tc.tile_pool(name="kraw", bufs=4) as kraw_p, \
             tc.tile_pool(name="qraw", bufs=2) as qraw_p, \
             tc.tile_pool(name="ktmp", bufs=2) as ktmp_p, \
             tc.tile_pool(name="kps", bufs=4, space="PSUM") as kps_p, \
             tc.tile_pool(name="qps", bufs=3, space="PSUM") as qps_p:
            # DMA issue order = order of first use: K block 0 only needs
            # wk+xb0 (8.7us of DMA) -> PE starts at ~11us; Q-chunk inputs
            # stream while K block 0 runs and fill later block boundaries.
            wk_sb = p1_p.tile([C, 8, D], BF16, tag="wk")
            nc.sync.dma_start(wk_sb[:], w_in[:, 8:16, :])
            xb0 = kstr_p.tile([C, 8, 512], BF16, tag="xb", name="xb_0")
            nc.sync.dma_start(xb0[:], xt[:, :, 0:512])
            cb0 = cstr_p.tile([C, 8, 512], BF16, tag="cb", name="cb_0")
            nc.sync.dma_start(cb0[:], csk[:, :, 0:512])
            wq_sb = p1_p.tile([C, 8, D], BF16, tag="wq")
            nc.sync.dma_start(wq_sb[:], w_in[:, 0:8, :])
            xq_sb = p1_p.tile([C, 8, NOR], BF16, tag="xq")
            nc.sync.dma_start(xq_sb[:, :, 0:2 * C], xq[:, :, 0:2 * C])
            for blk in range(NBLK):
                if blk == 1:
                    nc.sync.dma_start(xq_sb[:, :, 2 * C:NOR], xq[:, :, 2 * C:NOR])
                if blk == 2:
                    # wv/masks needed only in phase V; issue behind the early
                    # x blocks
                    nc.sync.dma_start(wv_sb[:], w_in[:, 16:24, :])
                    nc.sync.dma_start(masks_sb[:], masks)
                rows = slice(blk * 512, (blk + 1) * 512)
                if blk == 0:
                    xb, cb = xb0, cb0
                else:
                    xb = kstr_p.tile([C, 8, 512], BF16, tag="xb", name=f"xb_{blk}")
                    nc.sync.dma_start(xb[:], xt[:, :, rows])
                    cb = cstr_p.tile([C, 8, 512], BF16, tag="cb", name=f"cb_{blk}")
                    nc.sync.dma_start(cb[:], csk[:, :, rows])
                for pr in range(4):
                    kraw = []
                    for dc in (pr, pr + 4):
                        ps = kps_p.tile([C, 512], F32, tag="kps",
                                        name=f"kps_{blk}_{dc}")
                        for dcd in range(8):
                            nc.tensor.matmul(ps[:],
                                             wk_sb[:, dcd, dc * C:(dc + 1) * C],
                                             xb[:, dcd, :],
                                             start=(dcd == 0), stop=(dcd == 7))
                        kr = kraw_p.tile([C, 512], BF16, tag="kraw",
                                         name=f"kraw_{blk}_{dc}")
                        nc.scalar.copy(kr[:], ps[:])
                        kraw.append(kr)
                    re, im = kraw[0], kraw[1]
                    cc, ss = cb[:, pr, :], cb[:, pr + 4, :]
                    t0 = ktmp_p.tile([C, 512], BF16, tag="kt0", name=f"kt0_{blk}_{pr}")
                    t1 = ktmp_p.tile([C, 512], BF16, tag="kt1", name=f"kt1_{blk}_{pr}")
                    nc.vector.tensor_mul(t0[:], re[:], cc)
                    nc.vector.tensor_mul(t1[:], im[:], ss)
                    nc.vector.tensor_sub(kt_sb[:, pr, rows], t0[:], t1[:])
                    t2 = ktmp_p.tile([C, 512], BF16, tag="kt2", name=f"kt2_{blk}_{pr}")
                    t3 = ktmp_p.tile([C, 512], BF16, tag="kt3", name=f"kt3_{blk}_{pr}")
                    nc.vector.tensor_mul(t2[:], re[:], ss)
                    nc.vector.tensor_mul(t3[:], im[:], cc)
                    nc.vector.tensor_add(kt_sb[:, pr + 4, rows], t2[:], t3[:])

                _emit_qchunk(nc, qstr_p, qraw_p, ktmp_p, qps_p, csq,
                             wq_sb, xq_sb, qt_sb, blk)

        # ---- phase V + attention, interleaved per 512-block/slot ----
        v_p = ctx.enter_context(tc.tile_pool(name="v", bufs=1))
        v_sb = v_p.tile([C, NQC, D], BF16, tag="v")
        with tc.tile_pool(name="vstr", bufs=2) as vstr_p, \
             tc.tile_pool(name="pp", bufs=2) as pp_p, \
             tc.tile_pool(name="at", bufs=2) as at_p, \
             tc.tile_pool(name="vps", bufs=2, space="PSUM") as vps_p, \
             tc.tile_pool(name="scps", bufs=2, space="PSUM") as scps_p, \
             tc.tile_pool(name="ops", bufs=2, space="PSUM") as ops_p, \
             tc.tile_pool(name="lsps", bufs=2, space="PSUM") as lsps_p:

            def _emit_slot(t):
                # Scores computed TRANSPOSED (S^T[kv, q]) into 128-col slices
                # of a [C,512] PSUM quartet tile: exp output IS P^T (no PE
                # transposes), row sums via ones-matmul chains, mask appended
                # per-slice on the diagonal quartet.
                kpat = 0 if t < 4 else 1
                qc = slice(t * C, (t + 1) * C)
                p_sb = pp_p.tile([C, S], BF16, tag="p", name=f"p_{t}")
                ls_ps = lsps_p.tile([C, 1], F32, tag="lsps", name=f"ls_{t}")
                for u in range(t + 1):
                    cols = slice(u * 512, (u + 1) * 512)
                    sps = scps_p.tile([C, 512], F32, tag="scps",
                                      name=f"sps_{t}_{u}")
                    for c in range(4):
                        ch = 4 * u + c
                        csl = slice(c * C, (c + 1) * C)
                        diag = u == t
                        for dc in range(8):
                            nc.tensor.matmul(
                                sps[:, csl],
                                kt_sb[:, dc, ch * C:(ch + 1) * C],
                                qt_sb[:, dc, qc],
                                start=(dc == 0),
                                stop=(dc == 7 and not diag))
                        if diag:
                            nc.tensor.matmul(sps[:, csl], ident[:],
                                             masks_sb[:, kpat, csl],
                                             start=False, stop=True)
                    nc.scalar.activation(p_sb[:, cols], sps[:],
                                         mybir.ActivationFunctionType.Exp,
                                         scale=SCALE)
                    # lsum for quartet u-1 (exp already done -> no PE stall);
                    # quartet t's lsum lands after the loop
                    for ud in ([u - 1] if u >= 1 else []) + ([t] if u == t else []):
                        for c in range(4):
                            ch = 4 * ud + c
                            nc.tensor.matmul(ls_ps[:],
                                             p_sb[:, ch * C:(ch + 1) * C],
                                             ones_sb[:],
                                             start=(ud == 0 and c == 0),
                                             stop=(ud == t and c == 3))
                rinv = at_p.tile([C, 1], F32, tag="rinv", name=f"rinv_{t}")
                nc.vector.reciprocal(rinv[:], ls_ps[:])

                ob = at_p.tile([C, D], BF16, tag="ob", name=f"ob_{t}")
                # final slot: 256-wide output pieces shorten the
                # end-of-kernel scale+store chain
                nq = 4 if t == NOC - 1 else 2
                qw = D // nq
                for h in range(nq):
                    cols = slice(h * qw, (h + 1) * qw)
                    o_ps = ops_p.tile([C, qw], F32, tag="ops",
                                      name=f"ops_{t}_{h}")
                    for u in range(t + 1):
                        for j in range(4):
                            ch = 4 * u + j
                            nc.tensor.matmul(o_ps[:],
                                             p_sb[:, ch * C:(ch + 1) * C],
                                             v_sb[:, ch, cols],
                                             start=(u == 0 and j == 0),
                                             stop=(u == t and j == 3))
                    if h % 2 == 0:
                        nc.vector.tensor_scalar_mul(ob[:, cols], o_ps[:],
                                                    rinv[:])
                    else:
                        nc.scalar.mul(ob[:, cols], o_ps[:], rinv[:])
                    nc.sync.dma_start(o_fin[t, :, cols], ob[:, cols])

            for t in range(NOC):
                # V projection for kv block t (chunks 4t..4t+3)
                rows = slice(t * 512, (t + 1) * 512)
                xb = vstr_p.tile([C, 8, 512], BF16, tag="vxb", name=f"vxb_{t}")
                nc.sync.dma_start(xb[:], xt[:, :, rows])
                for c4 in range(4):
                    rsl = slice(c4 * C, (c4 + 1) * C)
                    for h in range(2):
                        cols = slice(h * 512, (h + 1) * 512)
                        vp = vps_p.tile([C, 512], F32, tag="vps",
                                        name=f"vps_{t}_{c4}_{h}")
                        for dc in range(8):
                            nc.tensor.matmul(vp[:], xb[:, dc, rsl],
                                             wv_sb[:, dc, cols],
                                             start=(dc == 0), stop=(dc == 7))
                        nc.scalar.copy(v_sb[:, 4 * t + c4, cols], vp[:])

                # attention slot t: q chunk QCH[i][t], kv cols [0, 512*(t+1))
                _emit_slot(t)

    nc.compile()
    return nc


def _xt_blocked(rows_x):
    """[n, D] float -> [C, 8, n] bf16 blocked transpose."""
    return np.ascontiguousarray(
        rows_x.astype(NPBF16).reshape(-1, 8, C).transpose(2, 1, 0))


def _masks(i):
    # transposed-score masks: tile [kv(part) p, q j] allows kv<=q -> p<=j
    tri = np.where(np.arange(C)[:, None] <= np.arange(C)[None, :], 0.0, NEG)
    m = np.zeros((C, 2, 512), np.float32)
    for k, diag in enumerate((i, 3 - i)):
        for c in range(4):
            if c > diag:
                m[:, k, c * C:(c + 1) * C] = NEG
            elif c == diag:
                m[:, k, c * C:(c + 1) * C] = tri
    return np.ascontiguousarray(m.astype(NPBF16))


def _prep_inputs(x, w_q, w_k, w_v, freqs_cos, freqs_sin):
    perm = np.concatenate([np.arange(0, D, 2), np.arange(1, D, 2)])
    wqT = np.ascontiguousarray(w_q[perm, :].T)
    wkT = np.ascontiguousarray(w_k[perm, :].T)
    wvT = np.ascontiguousarray(w_v.T)

    def blk(wt):  # [D, D] -> [C, dc, e]
        return wt.astype(NPBF16).reshape(8, C, D).transpose(1, 0, 2)

    flat24 = np.ascontiguousarray(
        np.concatenate([blk(wqT), blk(wkT), blk(wvT)], axis=1))  # [C, 24, D]

    cs_all = np.concatenate([freqs_cos, freqs_sin], axis=1)  # [S, D]
    csk_b = _xt_blocked(cs_all)                              # same for all cores
    xt_b = [_xt_blocked(np.asarray(x[b])) for b in range(B)]

    in_maps = []
    for core in range(8):
        b, i = divmod(core, 4)
        qrows = (np.asarray(QCH[i])[:, None] * C + np.arange(C)[None, :]).reshape(-1)
        in_maps.append({
            "xt": xt_b[b],
            "xq": _xt_blocked(np.asarray(x[b])[qrows]),
            "csq": _xt_blocked(cs_all[qrows]),
            "csk": csk_b,
            "w_in": flat24,
            "masks": _masks(i),
        })
    return in_maps


def _assemble(results):
    out = np.empty((B, S, D), np.float32)
    for core in range(8):
        b, i = divmod(core, 4)
        o = np.asarray(results[core]["o_fin"], np.float32)  # [NOC, C, D]
        for t, j in enumerate(QCH[i]):
            out[b, j * C:(j + 1) * C] = o[t]
    return out


def _run_pjrt(nc, in_maps, n_cores=8):
    """Like bass2jax.run_bass_via_pjrt, but creates the donated output
    buffers ON DEVICE (jit zeros) instead of uploading host zeros."""
    import jax
    import jax.numpy as jnp
    from jax.sharding import Mesh, NamedSharding, PartitionSpec
    try:
        from jax import shard_map
    except ImportError:
        from jax.experimental.shard_map import shard_map
    from concourse.bass2jax import (_bass_exec_p, install_neuronx_cc_hook,
                                    partition_id_tensor)

    install_neuronx_cc_hook()
    partition_name = nc.partition_id_tensor.name if nc.partition_id_tensor else None
    in_names, out_names, out_avals = [], [], []
    for alloc in nc.m.functions[0].allocations:
        if not isinstance(alloc, mybir.MemoryLocationSet):
            continue
        name = alloc.memorylocations[0].name
        if alloc.kind == "ExternalInput":
            if name != partition_name:
                in_names.append(name)
        elif alloc.kind == "ExternalOutput":
            out_avals.append(jax.core.ShapedArray(
                tuple(alloc.tensor_shape), mybir.dt.np(alloc.dtype)))
            out_names.append(name)
    n_params = len(in_names)
    all_in = list(in_names) + list(out_names)
    if partition_name is not None:
        all_in.append(partition_name)
    donate = tuple(range(n_params, n_params + len(out_names)))

    def _body(*args):
        operands = list(args)
        if partition_name is not None:
            operands.append(partition_id_tensor())
        return tuple(_bass_exec_p.bind(
            *operands, out_avals=tuple(out_avals), in_names=tuple(all_in),
            out_names=tuple(out_names), lowering_input_output_aliases=(),
            sim_require_finite=True, sim_require_nnan=True, nc=nc))

    devices = jax.devices()[:n_cores]
    mesh = Mesh(np.asarray(devices), ("core",))
    nio = n_params + len(out_names)
    sm_kw = dict(mesh=mesh, in_specs=(PartitionSpec("core"),) * nio,
                 out_specs=(PartitionSpec("core"),) * len(out_names))
    try:
        smapped = shard_map(_body, check_vma=False, **sm_kw)
    except TypeError:
        smapped = shard_map(_body, check_rep=False, **sm_kw)
    sharded = jax.jit(smapped, donate_argnums=donate, keep_unused=True)
    sh = NamedSharding(mesh, PartitionSpec("core"))
    zeros = jax.jit(
        lambda: tuple(jnp.zeros((n_cores * a.shape[0], *a.shape[1:]), a.dtype)
                      for a in out_avals),
        out_shardings=(sh,) * len(out_avals))()
    concat_in = [np.concatenate([np.asarray(m[k]) for m in in_maps], axis=0)
                 for k in in_names]
    outs = [np.asarray(o) for o in sharded(*concat_in, *zeros)]
    per_core = []
    for c in range(n_cores):
        d = {}
        for name, arr in zip(out_names, outs):
            s0 = arr.shape[0] // n_cores
            d[name] = arr[c * s0:(c + 1) * s0]
        per_core.append(d)
    return per_core


def kernel(x, w_q, w_k, w_v, freqs_cos, freqs_sin, _want_results=False, _trace=False):
    if "nc" not in _CACHE:
        _CACHE["nc"] = _build()
    nc = _CACHE["nc"]
    in_maps = _prep_inputs(np.asarray(x, np.float32), np.asarray(w_q, np.float32),
                           np.asarray(w_k, np.float32), np.asarray(w_v, np.float32),
                           np.asarray(freqs_cos, np.float32),
                           np.asarray(freqs_sin, np.float32))
    if _trace:
        kr = run_bass_kernel_spmd(nc, in_maps, core_ids=list(range(8)), trace=True)
        out = _assemble(kr.results)
        return (out, kr) if _want_results else out
    try:
        results = _run_pjrt(nc, in_maps)
    except Exception as e:
        print(f"kernel: _run_pjrt failed ({type(e).__name__}: {e}); "
              "falling back to run_bass_kernel_spmd", file=sys.stderr)
        kr = run_bass_kernel_spmd(nc, in_maps, core_ids=list(range(8)))
        results = kr.results
    out = _assemble(results)
    if _want_results:
        return out, results
    return out


# revision 53
# speedup vs baseline: 1.0457x; 1.0063x over previous
"""Causal single-head attention (B=2, S=4096, D=1024) + RoPE on 8 TRN2 cores.

Collective-free design: cores 4b+i (b=batch, i=rank 0..3) each receive the
FULL weights and the full x^T of their batch from the host, so no weight or
KV AllGather is needed.  Each core:

  1. projects+ropes Q^T for its own 8 q-chunks QCH[i] (balanced causal mass),
  2. projects K^T directly in transposed layout (stationary=W^T, moving=x^T)
     for ALL 4096 rows and ropes it in transposed layout,
  3. projects V naturally for all rows, interleaved with
  4. 8 attention slots: slot t attends the first 512*(t+1) kv columns; the
     causal edge mask is folded into the scores PSUM accumulation via one
     matmul (stationary=identity, moving=host-built mask), softmax runs
     without max-subtraction (|scores*scale| <= ~3, f32-exact), and exp reads
     PSUM tiles directly with accum_out partial row sums.

Output: final softmax-normalized rows (bf16); host scatters them.
"""

import sys

sys.path.insert(0, "/opt/trn_rl_repo")

import math
from contextlib import ExitStack

import ml_dtypes
import numpy as np

import concourse.bass as bass
import concourse.tile as tile
from concourse import bacc, mybir
from concourse.bass_utils import run_bass_kernel_spmd
from concourse.masks import make_identity

BF16 = mybir.dt.bfloat16
F32 = mybir.dt.float32
NPBF16 = ml_dtypes.bfloat16

B, S, D = 2, 4096, 1024
H = D // 2
C = 128
NQC = S // C                  # 32 chunks of 128 rows
NOC = 8                       # own q chunks per core
NOR = NOC * C                 # 1024 own q rows
NBLK = S // 512               # 8 512-row blocks
SCALE = 1.0 / math.sqrt(D)
NEG = -30000.0

QCH = [sorted([4 * t + i for t in range(4)] + [4 * t + 3 - i for t in range(4, 8)])
       for i in range(4)]

_CACHE = {}


def _emit_qchunk(nc, qstr_p, qraw_p, ktmp_p, qps_p, csq, wq_sb, xq_sb, qt_sb, blk):
    """Project (transposed layout) + rope own q chunk #blk."""
    qcols = slice(blk * C, (blk + 1) * C)
    cq = qstr_p.tile([C, 8, C], BF16, tag="cq", name=f"cq_{blk}")
    nc.sync.dma_start(cq[:], csq[:, :, qcols])
    qraw_c = qraw_p.tile([C, 8, C], BF16, tag="qraw", name=f"qraw_{blk}")
    for ec in range(8):
        qp = qps_p.tile([C, C], F32, tag="qps", name=f"qp_{blk}_{ec}")
        for dc in range(8):
            nc.tensor.matmul(qp[:], wq_sb[:, dc, ec * C:(ec + 1) * C],
                             xq_sb[:, dc, qcols],
                             start=(dc == 0), stop=(dc == 7))
        nc.scalar.copy(qraw_c[:, ec, :], qp[:])
    for pr in range(4):
        cc, ss = cq[:, pr, :], cq[:, pr + 4, :]
        re, im = qraw_c[:, pr, :], qraw_c[:, pr + 4, :]
        t0 = ktmp_p.tile([C, C], BF16, tag="qt0", name=f"qt0_{blk}_{pr}")
        t1 = ktmp_p.tile([C, C], BF16, tag="qt1", name=f"qt1_{blk}_{pr}")
        nc.vector.tensor_mul(t0[:], re, cc)
        nc.vector.tensor_mul(t1[:], im, ss)
        nc.vector.tensor_sub(qt_sb[:, pr, qcols], t0[:], t1[:])
        t2 = ktmp_p.tile([C, C], BF16, tag="qt2", name=f"qt2_{blk}_{pr}")
        t3 = ktmp_p.tile([C, C], BF16, tag="qt3", name=f"qt3_{blk}_{pr}")
        nc.vector.tensor_mul(t2[:], re, ss)
        nc.vector.tensor_mul(t3[:], im, cc)
        nc.vector.tensor_add(qt_sb[:, pr + 4, qcols], t2[:], t3[:])


def _build():
    nc = bacc.Bacc("TRN2", target_bir_lowering=False, debug=False,
                   enable_asserts=False, num_devices=8)

    xt = nc.dram_tensor("xt", [C, 8, S], BF16, kind="ExternalInput").ap()
    xq = nc.dram_tensor("xq", [C, 8, NOR], BF16, kind="ExternalInput").ap()
    csq = nc.dram_tensor("csq", [C, 8, NOR], BF16, kind="ExternalInput").ap()
    csk = nc.dram_tensor("csk", [C, 8, S], BF16, kind="ExternalInput").ap()
    w_in = nc.dram_tensor("w_in", [C, 24, D], BF16, kind="ExternalInput").ap()
    masks = nc.dram_tensor("masks", [C, 2, 512], BF16, kind="ExternalInput").ap()

    o_fin = nc.dram_tensor("o_fin", [NOC, C, D], BF16, kind="ExternalOutput").ap()

    with tile.TileContext(nc) as tc, ExitStack() as ctx:
        const_p = ctx.enter_context(tc.tile_pool(name="const", bufs=1))
        ident = const_p.tile([C, C], BF16)
        make_identity(nc, ident[:])
        masks_sb = const_p.tile([C, 2, 512], BF16)
        ones_sb = const_p.tile([C, 1], BF16)
        nc.vector.memset(ones_sb[:], 1.0)

        # PE warmup: ~11us of dummy transposes while the first weight/x DMAs
        # stream, so the tensor engine is at full p-state when real matmuls
        # start (the results are never read). Source is a fast DVE memset
        # tile so warmup starts before the Pool-built identity is ready.
        with tc.tile_pool(name="wup", bufs=1, space="PSUM") as wup_p, \
             tc.tile_pool(name="wsrc", bufs=1) as wsrc_p:
            wsrc = wsrc_p.tile([C, C], BF16, tag="wsrc")
            nc.vector.memset(wsrc[:], 0.0)
            wup = wup_p.tile([C, C], BF16, tag="wup")
            for r in range(118):
                nc.tensor.transpose(wup[:], wsrc[:], wsrc[:])

        qt_p = ctx.enter_context(tc.tile_pool(name="qt", bufs=1))
        qt_sb = qt_p.tile([C, 8, NOR], BF16, tag="qt")
        # resident until the end: K^T and the V-projection weights (prefetched)
        kt_p = ctx.enter_context(tc.tile_pool(name="kt", bufs=1))
        kt_sb = kt_p.tile([C, 8, S], BF16, tag="kt")
        wv_sb = kt_p.tile([C, 8, D], BF16, tag="wv")

        # ---- phase 1: streamed K^T projection+rope for all rows, with the
        #      own-q chunk of each block projected+roped along the way ----
        with tc.tile_pool(name="ph1", bufs=1) as p1_p, \
             tc.tile_pool(name="kstr", bufs=2) as kstr_p, \
             tc.tile_pool(name="cstr", bufs=2) as cstr_p, \
             tc.tile_pool(name="qstr", bufs=2) as qstr_p, \
             tc.tile_pool(name="kraw", bufs=4) as kraw_p, \
             tc.tile_pool(name="qraw", bufs=2) as qraw_p, \
             tc.tile_pool(name="ktmp", bufs=2) as ktmp_p, \
             tc.tile_pool(name="kps", bufs=4, space="PSUM") as kps_p, \
             tc.tile_pool(name="qps", bufs=3, space="PSUM") as qps_p:
            # DMA issue order = order of first use: K block 0 only needs
            # wk+xb0 (8.7us of DMA) -> PE starts at ~11us; Q-chunk inputs
            # stream while K block 0 runs and fill later block boundaries.
            wk_sb = p1_p.tile([C, 8, D], BF16, tag="wk")
            nc.sync.dma_start(wk_sb[:, 0:4, :], w_in[:, 8:12, :])
            xb0 = kstr_p.tile([C, 8, 512], BF16, tag="xb", name="xb_0")
            nc.sync.dma_start(xb0[:], xt[:, :, 0:512])
            nc.sync.dma_start(wk_sb[:, 4:8, :], w_in[:, 12:16, :])
            cb0 = cstr_p.tile([C, 8, 512], BF16, tag="cb", name="cb_0")
            nc.sync.dma_start(cb0[:], csk[:, :, 0:512])
            wq_sb = p1_p.tile([C, 8, D], BF16, tag="wq")
            nc.sync.dma_start(wq_sb[:], w_in[:, 0:8, :])
            xq_sb = p1_p.tile([C, 8, NOR], BF16, tag="xq")
            nc.sync.dma_start(xq_sb[:, :, 0:2 * C], xq[:, :, 0:2 * C])
            for blk in range(NBLK):
                if blk == 1:
                    nc.sync.dma_start(xq_sb[:, :, 2 * C:NOR], xq[:, :, 2 * C:NOR])
                if blk == 2:
                    # wv/masks needed only in phase V; issue behind the early
                    # x blocks
                    nc.sync.dma_start(wv_sb[:], w_in[:, 16:24, :])
                    nc.sync.dma_start(masks_sb[:], masks)
                rows = slice(blk * 512, (blk + 1) * 512)
                if blk == 0:
                    xb, cb = xb0, cb0
                else:
                    xb = kstr_p.tile([C, 8, 512], BF16, tag="xb", name=f"xb_{blk}")
                    nc.sync.dma_start(xb[:], xt[:, :, rows])
                    cb = cstr_p.tile([C, 8, 512], BF16, tag="cb", name=f"cb_{blk}")
                    nc.sync.dma_start(cb[:], csk[:, :, rows])
                for pr in range(4):
                    kraw = []
                    for dc in (pr, pr + 4):
                        ps = kps_p.tile([C, 512], F32, tag="kps",
                                        name=f"kps_{blk}_{dc}")
                        for dcd in range(8):
                            nc.tensor.matmul(ps[:],
                                             wk_sb[:, dcd, dc * C:(dc + 1) * C],
                                             xb[:, dcd, :],
                                             start=(dcd == 0), stop=(dcd == 7))
                        kr = kraw_p.tile([C, 512], BF16, tag="kraw",
                                         name=f"kraw_{blk}_{dc}")
                        nc.scalar.copy(kr[:], ps[:])
                        kraw.append(kr)
                    re, im = kraw[0], kraw[1]
                    cc, ss = cb[:, pr, :], cb[:, pr + 4, :]
                    t0 = ktmp_p.tile([C, 512], BF16, tag="kt0", name=f"kt0_{blk}_{pr}")
                    t1 = ktmp_p.tile([C, 512], BF16, tag="kt1", name=f"kt1_{blk}_{pr}")
                    nc.vector.tensor_mul(t0[:], re[:], cc)
                    nc.vector.tensor_mul(t1[:], im[:], ss)
                    nc.vector.tensor_sub(kt_sb[:, pr, rows], t0[:], t1[:])
                    t2 = ktmp_p.tile([C, 512], BF16, tag="kt2", name=f"kt2_{blk}_{pr}")
                    t3 = ktmp_p.tile([C, 512], BF16, tag="kt3", name=f"kt3_{blk}_{pr}")
                    nc.vector.tensor_mul(t2[:], re[:], ss)
                    nc.vector.tensor_mul(t3[:], im[:], cc)
                    nc.vector.tensor_add(kt_sb[:, pr + 4, rows], t2[:], t3[:])

                _emit_qchunk(nc, qstr_p, qraw_p, ktmp_p, qps_p, csq,
                             wq_sb, xq_sb, qt_sb, blk)

        # ---- phase V + attention, interleaved per 512-block/slot ----
        v_p = ctx.enter_context(tc.tile_pool(name="v", bufs=1))
        v_sb = v_p.tile([C, NQC, D], BF16, tag="v")
        with tc.tile_pool(name="vstr", bufs=2) as vstr_p, \
             tc.tile_pool(name="pp", bufs=2) as pp_p, \
             tc.tile_pool(name="at", bufs=2) as at_p, \
             tc.tile_pool(name="vps", bufs=2, space="PSUM") as vps_p, \
             tc.tile_pool(name="scps", bufs=2, space="PSUM") as scps_p, \
             tc.tile_pool(name="ops", bufs=2, space="PSUM") as ops_p, \
             tc.tile_pool(name="lsps", bufs=2, space="PSUM") as lsps_p:

            def _emit_slot(t):
                # Scores computed TRANSPOSED (S^T[kv, q]) into 128-col slices
                # of a [C,512] PSUM quartet tile: exp output IS P^T (no PE
                # transposes), row sums via ones-matmul chains, mask appended
                # per-slice on the diagonal quartet.
                kpat = 0 if t < 4 else 1
                qc = slice(t * C, (t + 1) * C)
                p_sb = pp_p.tile([C, S], BF16, tag="p", name=f"p_{t}")
                ls_ps = lsps_p.tile([C, 1], F32, tag="lsps", name=f"ls_{t}")
                for u in range(t + 1):
                    cols = slice(u * 512, (u + 1) * 512)
                    sps = scps_p.tile([C, 512], F32, tag="scps",
                                      name=f"sps_{t}_{u}")
                    for c in range(4):
                        ch = 4 * u + c
                        csl = slice(c * C, (c + 1) * C)
                        diag = u == t
                        for dc in range(8):
                            nc.tensor.matmul(
                                sps[:, csl],
                                kt_sb[:, dc, ch * C:(ch + 1) * C],
                                qt_sb[:, dc, qc],
                                start=(dc == 0),
                                stop=(dc == 7 and not diag))
                        if diag:
                            nc.tensor.matmul(sps[:, csl], ident[:],
                                             masks_sb[:, kpat, csl],
                                             start=False, stop=True)
                    nc.scalar.activation(p_sb[:, cols], sps[:],
                                         mybir.ActivationFunctionType.Exp,
                                         scale=SCALE)
                    # lsum for quartet u-1 (exp already done -> no PE stall);
                    # quartet t's lsum lands after the loop
                    for ud in ([u - 1] if u >= 1 else []) + ([t] if u == t else []):
                        for c in range(4):
                            ch = 4 * ud + c
                            nc.tensor.matmul(ls_ps[:],
                                             p_sb[:, ch * C:(ch + 1) * C],
                                             ones_sb[:],
                                             start=(ud == 0 and c == 0),
                                             stop=(ud == t and c == 3))
                rinv = at_p.tile([C, 1], F32, tag="rinv", name=f"rinv_{t}")
                nc.vector.reciprocal(rinv[:], ls_ps[:])

                ob = at_p.tile([C, D], BF16, tag="ob", name=f"ob_{t}")
                # final slot: 256-wide output pieces shorten the
                # end-of-kernel scale+store chain
                nq = 4 if t == NOC - 1 else 2
                qw = D // nq
                for h in range(nq):
                    cols = slice(h * qw, (h + 1) * qw)
                    o_ps = ops_p.tile([C, qw], F32, tag="ops",
                                      name=f"ops_{t}_{h}")
                    for u in range(t + 1):
                        for j in range(4):
                            ch = 4 * u + j
                            nc.tensor.matmul(o_ps[:],
                                             p_sb[:, ch * C:(ch + 1) * C],
                                             v_sb[:, ch, cols],
                                             start=(u == 0 and j == 0),
                                             stop=(u == t and j == 3))
                    if h % 2 == 0:
                        nc.vector.tensor_scalar_mul(ob[:, cols], o_ps[:],
                                                    rinv[:])
                    else:
                        nc.scalar.mul(ob[:, cols], o_ps[:], rinv[:])
                    nc.sync.dma_start(o_fin[t, :, cols], ob[:, cols])

            for t in range(NOC):
                # V projection for kv block t (chunks 4t..4t+3)
                rows = slice(t * 512, (t + 1) * 512)
                xb = vstr_p.tile([C, 8, 512], BF16, tag="vxb", name=f"vxb_{t}")
                nc.sync.dma_start(xb[:], xt[:, :, rows])
                for c4 in range(4):
                    rsl = slice(c4 * C, (c4 + 1) * C)
                    for h in range(2):
                        cols = slice(h * 512, (h + 1) * 512)
                        vp = vps_p.tile([C, 512], F32, tag="vps",
                                        name=f"vps_{t}_{c4}_{h}")
                        for dc in range(8):
                            nc.tensor.matmul(vp[:], xb[:, dc, rsl],
                                             wv_sb[:, dc, cols],
                                             start=(dc == 0), stop=(dc == 7))
                        nc.scalar.copy(v_sb[:, 4 * t + c4, cols], vp[:])

                # attention slot t: q chunk QCH[i][t], kv cols [0, 512*(t+1))
                _emit_slot(t)

    nc.compile()
    return nc


def _xt_blocked(rows_x):
    """[n, D] float -> [C, 8, n] bf16 blocked transpose."""
    return np.ascontiguousarray(
        rows_x.astype(NPBF16).reshape(-1, 8, C).transpose(2, 1, 0))


def _masks(i):
    # transposed-score masks: tile [kv(part) p, q j] allows kv<=q -> p<=j
    tri = np.where(np.arange(C)[:, None] <= np.arange(C)[None, :], 0.0, NEG)
    m = np.zeros((C, 2, 512), np.float32)
    for k, diag in enumerate((i, 3 - i)):
        for c in range(4):
            if c > diag:
                m[:, k, c * C:(c + 1) * C] = NEG
            elif c == diag:
                m[:, k, c * C:(c + 1) * C] = tri
    return np.ascontiguousarray(m.astype(NPBF16))


def _prep_inputs(x, w_q, w_k, w_v, freqs_cos, freqs_sin):
    perm = np.concatenate([np.arange(0, D, 2), np.arange(1, D, 2)])
    wqT = np.ascontiguousarray(w_q[perm, :].T)
    wkT = np.ascontiguousarray(w_k[perm, :].T)
    wvT = np.ascontiguousarray(w_v.T)

    def blk(wt):  # [D, D] -> [C, dc, e]
        return wt.astype(NPBF16).reshape(8, C, D).transpose(1, 0, 2)

    flat24 = np.ascontiguousarray(
        np.concatenate([blk(wqT), blk(wkT), blk(wvT)], axis=1))  # [C, 24, D]

    cs_all = np.concatenate([freqs_cos, freqs_sin], axis=1)  # [S, D]
    csk_b = _xt_blocked(cs_all)                              # same for all cores
    xt_b = [_xt_blocked(np.asarray(x[b])) for b in range(B)]

    in_maps = []
    for core in range(8):
        b, i = divmod(core, 4)
        qrows = (np.asarray(QCH[i])[:, None] * C + np.arange(C)[None, :]).reshape(-1)
        in_maps.append({
            "xt": xt_b[b],
            "xq": _xt_blocked(np.asarray(x[b])[qrows]),
            "csq": _xt_blocked(cs_all[qrows]),
            "csk": csk_b,
            "w_in": flat24,
            "masks": _masks(i),
        })
    return in_maps


def _assemble(results):
    out = np.empty((B, S, D), np.float32)
    for core in range(8):
        b, i = divmod(core, 4)
        o = np.asarray(results[core]["o_fin"], np.float32)  # [NOC, C, D]
        for t, j in enumerate(QCH[i]):
            out[b, j * C:(j + 1) * C] = o[t]
    return out


def _run_pjrt(nc, in_maps, n_cores=8):
    """Like bass2jax.run_bass_via_pjrt, but creates the donated output
    buffers ON DEVICE (jit zeros) instead of uploading host zeros."""
    import jax
    import jax.numpy as jnp
    from jax.sharding import Mesh, NamedSharding, PartitionSpec
    try:
        from jax import shard_map
    except ImportError:
        from jax.experimental.shard_map import shard_map
    from concourse.bass2jax import (_bass_exec_p, install_neuronx_cc_hook,
                                    partition_id_tensor)

    install_neuronx_cc_hook()
    partition_name = nc.partition_id_tensor.name if nc.partition_id_tensor else None
    in_names, out_names, out_avals = [], [], []
    for alloc in nc.m.functions[0].allocations:
        if not isinstance(alloc, mybir.MemoryLocationSet):
            continue
        name = alloc.memorylocations[0].name
        if alloc.kind == "ExternalInput":
            if name != partition_name:
                in_names.append(name)
        elif alloc.kind == "ExternalOutput":
            out_avals.append(jax.core.ShapedArray(
                tuple(alloc.tensor_shape), mybir.dt.np(alloc.dtype)))
            out_names.append(name)
    n_params = len(in_names)
    all_in = list(in_names) + list(out_names)
    if partition_name is not None:
        all_in.append(partition_name)
    donate = tuple(range(n_params, n_params + len(out_names)))

    def _body(*args):
        operands = list(args)
        if partition_name is not None:
            operands.append(partition_id_tensor())
        return tuple(_bass_exec_p.bind(
            *operands, out_avals=tuple(out_avals), in_names=tuple(all_in),
            out_names=tuple(out_names), lowering_input_output_aliases=(),
            sim_require_finite=True, sim_require_nnan=True, nc=nc))

    devices = jax.devices()[:n_cores]
    mesh = Mesh(np.asarray(devices), ("core",))
    nio = n_params + len(out_names)
    sm_kw = dict(mesh=mesh, in_specs=(PartitionSpec("core"),) * nio,
                 out_specs=(PartitionSpec("core"),) * len(out_names))
    try:
        smapped = shard_map(_body, check_vma=False, **sm_kw)
    except TypeError:
        smapped = shard_map(_body, check_rep=False, **sm_kw)
    sharded = jax.jit(smapped, donate_argnums=donate, keep_unused=True)
    sh = NamedSharding(mesh, PartitionSpec("core"))
    zeros = jax.jit(
        lambda: tuple(jnp.zeros((n_cores * a.shape[0], *a.shape[1:]), a.dtype)
                      for a in out_avals),
        out_shardings=(sh,) * len(out_avals))()
    concat_in = [np.concatenate([np.asarray(m[k]) for m in in_maps], axis=0)
                 for k in in_names]
    outs = [np.asarray(o) for o in sharded(*concat_in, *zeros)]
    per_core = []
    for c in range(n_cores):
        d = {}
        for name, arr in zip(out_names, outs):
            s0 = arr.shape[0] // n_cores
            d[name] = arr[c * s0:(c + 1) * s0]
        per_core.append(d)
    return per_core


def kernel(x, w_q, w_k, w_v, freqs_cos, freqs_sin, _want_results=False, _trace=False):
    if "nc" not in _CACHE:
        _CACHE["nc"] = _build()
    nc = _CACHE["nc"]
    in_maps = _prep_inputs(np.asarray(x, np.float32), np.asarray(w_q, np.float32),
                           np.asarray(w_k, np.float32), np.asarray(w_v, np.float32),
                           np.asarray(freqs_cos, np.float32),
                           np.asarray(freqs_sin, np.float32))
    if _trace:
        kr = run_bass_kernel_spmd(nc, in_maps, core_ids=list(range(8)), trace=True)
        out = _assemble(kr.results)
        return (out, kr) if _want_results else out
    try:
        results = _run_pjrt(nc, in_maps)
    except Exception as e:
        print(f"kernel: _run_pjrt failed ({type(e).__name__}: {e}); "
              "falling back to run_bass_kernel_spmd", file=sys.stderr)
        kr = run_bass_kernel_spmd(nc, in_maps, core_ids=list(range(8)))
        results = kr.results
    out = _assemble(results)
    if _want_results:
        return out, results
    return out


# revision 54
# speedup vs baseline: 1.0461x; 1.0003x over previous
"""Causal single-head attention (B=2, S=4096, D=1024) + RoPE on 8 TRN2 cores.

Collective-free design: cores 4b+i (b=batch, i=rank 0..3) each receive the
FULL weights and the full x^T of their batch from the host, so no weight or
KV AllGather is needed.  Each core:

  1. projects+ropes Q^T for its own 8 q-chunks QCH[i] (balanced causal mass),
  2. projects K^T directly in transposed layout (stationary=W^T, moving=x^T)
     for ALL 4096 rows and ropes it in transposed layout,
  3. projects V naturally for all rows, interleaved with
  4. 8 attention slots: slot t attends the first 512*(t+1) kv columns; the
     causal edge mask is folded into the scores PSUM accumulation via one
     matmul (stationary=identity, moving=host-built mask), softmax runs
     without max-subtraction (|scores*scale| <= ~3, f32-exact), and exp reads
     PSUM tiles directly with accum_out partial row sums.

Output: final softmax-normalized rows (bf16); host scatters them.
"""

import sys

sys.path.insert(0, "/opt/trn_rl_repo")

import math
from contextlib import ExitStack

import ml_dtypes
import numpy as np

import concourse.bass as bass
import concourse.tile as tile
from concourse import bacc, mybir
from concourse.bass_utils import run_bass_kernel_spmd
from concourse.masks import make_identity

BF16 = mybir.dt.bfloat16
F32 = mybir.dt.float32
NPBF16 = ml_dtypes.bfloat16

B, S, D = 2, 4096, 1024
H = D // 2
C = 128
NQC = S // C                  # 32 chunks of 128 rows
NOC = 8                       # own q chunks per core
NOR = NOC * C                 # 1024 own q rows
NBLK = S // 512               # 8 512-row blocks
SCALE = 1.0 / math.sqrt(D)
NEG = -30000.0

QCH = [sorted([4 * t + i for t in range(4)] + [4 * t + 3 - i for t in range(4, 8)])
       for i in range(4)]

_CACHE = {}


def _emit_qchunk(nc, qstr_p, qraw_p, ktmp_p, qps_p, csq, wq_sb, xq_sb, qt_sb, blk):
    """Project (transposed layout) + rope own q chunk #blk."""
    qcols = slice(blk * C, (blk + 1) * C)
    cq = qstr_p.tile([C, 8, C], BF16, tag="cq", name=f"cq_{blk}")
    nc.sync.dma_start(cq[:], csq[:, :, qcols])
    qraw_c = qraw_p.tile([C, 8, C], BF16, tag="qraw", name=f"qraw_{blk}")
    for ec in range(8):
        qp = qps_p.tile([C, C], F32, tag="qps", name=f"qp_{blk}_{ec}")
        for dc in range(8):
            nc.tensor.matmul(qp[:], wq_sb[:, dc, ec * C:(ec + 1) * C],
                             xq_sb[:, dc, qcols],
                             start=(dc == 0), stop=(dc == 7))
        nc.scalar.copy(qraw_c[:, ec, :], qp[:])
    for pr in range(4):
        cc, ss = cq[:, pr, :], cq[:, pr + 4, :]
        re, im = qraw_c[:, pr, :], qraw_c[:, pr + 4, :]
        t0 = ktmp_p.tile([C, C], BF16, tag="qt0", name=f"qt0_{blk}_{pr}")
        t1 = ktmp_p.tile([C, C], BF16, tag="qt1", name=f"qt1_{blk}_{pr}")
        nc.vector.tensor_mul(t0[:], re, cc)
        nc.vector.tensor_mul(t1[:], im, ss)
        nc.vector.tensor_sub(qt_sb[:, pr, qcols], t0[:], t1[:])
        t2 = ktmp_p.tile([C, C], BF16, tag="qt2", name=f"qt2_{blk}_{pr}")
        t3 = ktmp_p.tile([C, C], BF16, tag="qt3", name=f"qt3_{blk}_{pr}")
        nc.vector.tensor_mul(t2[:], re, ss)
        nc.vector.tensor_mul(t3[:], im, cc)
        nc.vector.tensor_add(qt_sb[:, pr + 4, qcols], t2[:], t3[:])


def _build():
    nc = bacc.Bacc("TRN2", target_bir_lowering=False, debug=False,
                   enable_asserts=False, num_devices=8)

    xt = nc.dram_tensor("xt", [C, 8, S], BF16, kind="ExternalInput").ap()
    xq = nc.dram_tensor("xq", [C, 8, NOR], BF16, kind="ExternalInput").ap()
    csq = nc.dram_tensor("csq", [C, 8, NOR], BF16, kind="ExternalInput").ap()
    csk = nc.dram_tensor("csk", [C, 8, S], BF16, kind="ExternalInput").ap()
    w_in = nc.dram_tensor("w_in", [C, 24, D], BF16, kind="ExternalInput").ap()
    masks = nc.dram_tensor("masks", [C, 2, 512], BF16, kind="ExternalInput").ap()

    o_fin = nc.dram_tensor("o_fin", [NOC, C, D], BF16, kind="ExternalOutput").ap()

    with tile.TileContext(nc) as tc, ExitStack() as ctx:
        const_p = ctx.enter_context(tc.tile_pool(name="const", bufs=1))
        ident = const_p.tile([C, C], BF16)
        make_identity(nc, ident[:])
        masks_sb = const_p.tile([C, 2, 512], BF16)
        ones_sb = const_p.tile([C, 1], BF16)
        nc.vector.memset(ones_sb[:], 1.0)

        # PE warmup: ~11us of dummy transposes while the first weight/x DMAs
        # stream, so the tensor engine is at full p-state when real matmuls
        # start (the results are never read). Source is a fast DVE memset
        # tile so warmup starts before the Pool-built identity is ready.
        with tc.tile_pool(name="wup", bufs=1, space="PSUM") as wup_p, \
             tc.tile_pool(name="wsrc", bufs=1) as wsrc_p:
            wsrc = wsrc_p.tile([C, C], BF16, tag="wsrc")
            nc.vector.memset(wsrc[:], 0.0)
            wup = wup_p.tile([C, C], BF16, tag="wup")
            for r in range(118):
                nc.tensor.transpose(wup[:], wsrc[:], wsrc[:])

        qt_p = ctx.enter_context(tc.tile_pool(name="qt", bufs=1))
        qt_sb = qt_p.tile([C, 8, NOR], BF16, tag="qt")
        # resident until the end: K^T and the V-projection weights (prefetched)
        kt_p = ctx.enter_context(tc.tile_pool(name="kt", bufs=1))
        kt_sb = kt_p.tile([C, 8, S], BF16, tag="kt")
        wv_sb = kt_p.tile([C, 8, D], BF16, tag="wv")

        # ---- phase 1: streamed K^T projection+rope for all rows, with the
        #      own-q chunk of each block projected+roped along the way ----
        with tc.tile_pool(name="ph1", bufs=1) as p1_p, \
             tc.tile_pool(name="kstr", bufs=2) as kstr_p, \
             tc.tile_pool(name="cstr", bufs=2) as cstr_p, \
             tc.tile_pool(name="qstr", bufs=2) as qstr_p, \
             tc.tile_pool(name="kraw", bufs=4) as kraw_p, \
             tc.tile_pool(name="qraw", bufs=2) as qraw_p, \
             tc.tile_pool(name="ktmp", bufs=2) as ktmp_p, \
             tc.tile_pool(name="kps", bufs=4, space="PSUM") as kps_p, \
             tc.tile_pool(name="qps", bufs=3, space="PSUM") as qps_p:
            # DMA issue order = order of first use: K block 0 only needs
            # wk+xb0 (8.7us of DMA) -> PE starts at ~11us; Q-chunk inputs
            # stream while K block 0 runs and fill later block boundaries.
            wk_sb = p1_p.tile([C, 8, D], BF16, tag="wk")
            nc.sync.dma_start(wk_sb[:, 0:4, :], w_in[:, 8:12, :])
            xb0 = kstr_p.tile([C, 8, 512], BF16, tag="xb", name="xb_0")
            nc.sync.dma_start(xb0[:], xt[:, :, 0:512])
            nc.sync.dma_start(wk_sb[:, 4:8, :], w_in[:, 12:16, :])
            cb0 = cstr_p.tile([C, 8, 512], BF16, tag="cb", name="cb_0")
            nc.sync.dma_start(cb0[:], csk[:, :, 0:512])
            wq_sb = p1_p.tile([C, 8, D], BF16, tag="wq")
            nc.sync.dma_start(wq_sb[:], w_in[:, 0:8, :])
            xq_sb = p1_p.tile([C, 8, NOR], BF16, tag="xq")
            nc.sync.dma_start(xq_sb[:, :, 0:2 * C], xq[:, :, 0:2 * C])
            for blk in range(NBLK):
                if blk == 1:
                    nc.sync.dma_start(xq_sb[:, :, 2 * C:NOR], xq[:, :, 2 * C:NOR])
                if blk == 2:
                    # wv/masks needed only in phase V; issue behind the early
                    # x blocks
                    nc.sync.dma_start(wv_sb[:], w_in[:, 16:24, :])
                    nc.sync.dma_start(masks_sb[:], masks)
                rows = slice(blk * 512, (blk + 1) * 512)
                if blk == 0:
                    xb, cb = xb0, cb0
                else:
                    xb = kstr_p.tile([C, 8, 512], BF16, tag="xb", name=f"xb_{blk}")
                    nc.sync.dma_start(xb[:], xt[:, :, rows])
                    cb = cstr_p.tile([C, 8, 512], BF16, tag="cb", name=f"cb_{blk}")
                    nc.sync.dma_start(cb[:], csk[:, :, rows])
                for pr in range(4):
                    kraw = []
                    for dc in (pr, pr + 4):
                        ps = kps_p.tile([C, 512], F32, tag="kps",
                                        name=f"kps_{blk}_{dc}")
                        for dcd in range(8):
                            nc.tensor.matmul(ps[:],
                                             wk_sb[:, dcd, dc * C:(dc + 1) * C],
                                             xb[:, dcd, :],
                                             start=(dcd == 0), stop=(dcd == 7))
                        kr = kraw_p.tile([C, 512], BF16, tag="kraw",
                                         name=f"kraw_{blk}_{dc}")
                        nc.scalar.copy(kr[:], ps[:])
                        kraw.append(kr)
                    re, im = kraw[0], kraw[1]
                    cc, ss = cb[:, pr, :], cb[:, pr + 4, :]
                    t0 = ktmp_p.tile([C, 512], BF16, tag="kt0", name=f"kt0_{blk}_{pr}")
                    t1 = ktmp_p.tile([C, 512], BF16, tag="kt1", name=f"kt1_{blk}_{pr}")
                    nc.vector.tensor_mul(t0[:], re[:], cc)
                    nc.vector.tensor_mul(t1[:], im[:], ss)
                    nc.vector.tensor_sub(kt_sb[:, pr, rows], t0[:], t1[:])
                    t2 = ktmp_p.tile([C, 512], BF16, tag="kt2", name=f"kt2_{blk}_{pr}")
                    t3 = ktmp_p.tile([C, 512], BF16, tag="kt3", name=f"kt3_{blk}_{pr}")
                    nc.vector.tensor_mul(t2[:], re[:], ss)
                    nc.vector.tensor_mul(t3[:], im[:], cc)
                    nc.vector.tensor_add(kt_sb[:, pr + 4, rows], t2[:], t3[:])

                _emit_qchunk(nc, qstr_p, qraw_p, ktmp_p, qps_p, csq,
                             wq_sb, xq_sb, qt_sb, blk)

        # ---- phase V + attention, interleaved per 512-block/slot ----
        v_p = ctx.enter_context(tc.tile_pool(name="v", bufs=1))
        v_sb = v_p.tile([C, NQC, D], BF16, tag="v")
        with tc.tile_pool(name="vstr", bufs=2) as vstr_p, \
             tc.tile_pool(name="pp", bufs=2) as pp_p, \
             tc.tile_pool(name="at", bufs=2) as at_p, \
             tc.tile_pool(name="vps", bufs=2, space="PSUM") as vps_p, \
             tc.tile_pool(name="scps", bufs=2, space="PSUM") as scps_p, \
             tc.tile_pool(name="ops", bufs=2, space="PSUM") as ops_p, \
             tc.tile_pool(name="lsps", bufs=2, space="PSUM") as lsps_p:

            def _emit_slot(t):
                # Scores computed TRANSPOSED (S^T[kv, q]) into 128-col slices
                # of a [C,512] PSUM quartet tile: exp output IS P^T (no PE
                # transposes), row sums via ones-matmul chains, mask appended
                # per-slice on the diagonal quartet.
                kpat = 0 if t < 4 else 1
                qc = slice(t * C, (t + 1) * C)
                p_sb = pp_p.tile([C, S], BF16, tag="p", name=f"p_{t}")
                ls_ps = lsps_p.tile([C, 1], F32, tag="lsps", name=f"ls_{t}")
                for u in range(t + 1):
                    cols = slice(u * 512, (u + 1) * 512)
                    sps = scps_p.tile([C, 512], F32, tag="scps",
                                      name=f"sps_{t}_{u}")
                    for c in range(4):
                        ch = 4 * u + c
                        csl = slice(c * C, (c + 1) * C)
                        diag = u == t
                        for dc in range(8):
                            nc.tensor.matmul(
                                sps[:, csl],
                                kt_sb[:, dc, ch * C:(ch + 1) * C],
                                qt_sb[:, dc, qc],
                                start=(dc == 0),
                                stop=(dc == 7 and not diag))
                        if diag:
                            nc.tensor.matmul(sps[:, csl], ident[:],
                                             masks_sb[:, kpat, csl],
                                             start=False, stop=True)
                    nc.scalar.activation(p_sb[:, cols], sps[:],
                                         mybir.ActivationFunctionType.Exp,
                                         scale=SCALE)
                    # lsum for quartet u-1 (exp already done -> no PE stall);
                    # quartet t's lsum lands after the loop
                    for ud in ([u - 1] if u >= 1 else []) + ([t] if u == t else []):
                        for c in range(4):
                            ch = 4 * ud + c
                            nc.tensor.matmul(ls_ps[:],
                                             p_sb[:, ch * C:(ch + 1) * C],
                                             ones_sb[:],
                                             start=(ud == 0 and c == 0),
                                             stop=(ud == t and c == 3))
                rinv = at_p.tile([C, 1], F32, tag="rinv", name=f"rinv_{t}")
                nc.vector.reciprocal(rinv[:], ls_ps[:])

                ob = at_p.tile([C, D], BF16, tag="ob", name=f"ob_{t}")
                # final slot: tapered output pieces shorten the
                # end-of-kernel scale+store chain
                widths = [256, 256, 256, 128, 128] if t == NOC - 1 else [512, 512]
                off = 0
                for h, qw in enumerate(widths):
                    cols = slice(off, off + qw)
                    off += qw
                    o_ps = ops_p.tile([C, qw], F32, tag="ops",
                                      name=f"ops_{t}_{h}")
                    for u in range(t + 1):
                        for j in range(4):
                            ch = 4 * u + j
                            nc.tensor.matmul(o_ps[:],
                                             p_sb[:, ch * C:(ch + 1) * C],
                                             v_sb[:, ch, cols],
                                             start=(u == 0 and j == 0),
                                             stop=(u == t and j == 3))
                    if h % 2 == 0:
                        nc.vector.tensor_scalar_mul(ob[:, cols], o_ps[:],
                                                    rinv[:])
                    else:
                        nc.scalar.mul(ob[:, cols], o_ps[:], rinv[:])
                    nc.sync.dma_start(o_fin[t, :, cols], ob[:, cols])

            for t in range(NOC):
                # V projection for kv block t (chunks 4t..4t+3)
                rows = slice(t * 512, (t + 1) * 512)
                xb = vstr_p.tile([C, 8, 512], BF16, tag="vxb", name=f"vxb_{t}")
                nc.sync.dma_start(xb[:], xt[:, :, rows])
                for c4 in range(4):
                    rsl = slice(c4 * C, (c4 + 1) * C)
                    for h in range(2):
                        cols = slice(h * 512, (h + 1) * 512)
                        vp = vps_p.tile([C, 512], F32, tag="vps",
                                        name=f"vps_{t}_{c4}_{h}")
                        for dc in range(8):
                            nc.tensor.matmul(vp[:], xb[:, dc, rsl],
                                             wv_sb[:, dc, cols],
                                             start=(dc == 0), stop=(dc == 7))
                        nc.scalar.copy(v_sb[:, 4 * t + c4, cols], vp[:])

                # attention slot t: q chunk QCH[i][t], kv cols [0, 512*(t+1))
                _emit_slot(t)

    nc.compile()
    return nc


def _xt_blocked(rows_x):
    """[n, D] float -> [C, 8, n] bf16 blocked transpose."""
    return np.ascontiguousarray(
        rows_x.astype(NPBF16).reshape(-1, 8, C).transpose(2, 1, 0))


def _masks(i):
    # transposed-score masks: tile [kv(part) p, q j] allows kv<=q -> p<=j
    tri = np.where(np.arange(C)[:, None] <= np.arange(C)[None, :], 0.0, NEG)
    m = np.zeros((C, 2, 512), np.float32)
    for k, diag in enumerate((i, 3 - i)):
        for c in range(4):
            if c > diag:
                m[:, k, c * C:(c + 1) * C] = NEG
            elif c == diag:
                m[:, k, c * C:(c + 1) * C] = tri
    return np.ascontiguousarray(m.astype(NPBF16))


def _prep_inputs(x, w_q, w_k, w_v, freqs_cos, freqs_sin):
    perm = np.concatenate([np.arange(0, D, 2), np.arange(1, D, 2)])
    wqT = np.ascontiguousarray(w_q[perm, :].T)
    wkT = np.ascontiguousarray(w_k[perm, :].T)
    wvT = np.ascontiguousarray(w_v.T)

    def blk(wt):  # [D, D] -> [C, dc, e]
        return wt.astype(NPBF16).reshape(8, C, D).transpose(1, 0, 2)

    flat24 = np.ascontiguousarray(
        np.concatenate([blk(wqT), blk(wkT), blk(wvT)], axis=1))  # [C, 24, D]

    cs_all = np.concatenate([freqs_cos, freqs_sin], axis=1)  # [S, D]
    csk_b = _xt_blocked(cs_all)                              # same for all cores
    xt_b = [_xt_blocked(np.asarray(x[b])) for b in range(B)]

    in_maps = []
    for core in range(8):
        b, i = divmod(core, 4)
        qrows = (np.asarray(QCH[i])[:, None] * C + np.arange(C)[None, :]).reshape(-1)
        in_maps.append({
            "xt": xt_b[b],
            "xq": _xt_blocked(np.asarray(x[b])[qrows]),
            "csq": _xt_blocked(cs_all[qrows]),
            "csk": csk_b,
            "w_in": flat24,
            "masks": _masks(i),
        })
    return in_maps


def _assemble(results):
    out = np.empty((B, S, D), np.float32)
    for core in range(8):
        b, i = divmod(core, 4)
        o = np.asarray(results[core]["o_fin"], np.float32)  # [NOC, C, D]
        for t, j in enumerate(QCH[i]):
            out[b, j * C:(j + 1) * C] = o[t]
    return out


def _run_pjrt(nc, in_maps, n_cores=8):
    """Like bass2jax.run_bass_via_pjrt, but creates the donated output
    buffers ON DEVICE (jit zeros) instead of uploading host zeros."""
    import jax
    import jax.numpy as jnp
    from jax.sharding import Mesh, NamedSharding, PartitionSpec
    try:
        from jax import shard_map
    except ImportError:
        from jax.experimental.shard_map import shard_map
    from concourse.bass2jax import (_bass_exec_p, install_neuronx_cc_hook,
                                    partition_id_tensor)

    install_neuronx_cc_hook()
    partition_name = nc.partition_id_tensor.name if nc.partition_id_tensor else None
    in_names, out_names, out_avals = [], [], []
    for alloc in nc.m.functions[0].allocations:
        if not isinstance(alloc, mybir.MemoryLocationSet):
            continue
        name = alloc.memorylocations[0].name
        if alloc.kind == "ExternalInput":
            if name != partition_name:
                in_names.append(name)
        elif alloc.kind == "ExternalOutput":
            out_avals.append(jax.core.ShapedArray(
                tuple(alloc.tensor_shape), mybir.dt.np(alloc.dtype)))
            out_names.append(name)
    n_params = len(in_names)
    all_in = list(in_names) + list(out_names)
    if partition_name is not None:
        all_in.append(partition_name)
    donate = tuple(range(n_params, n_params + len(out_names)))

    def _body(*args):
        operands = list(args)
        if partition_name is not None:
            operands.append(partition_id_tensor())
        return tuple(_bass_exec_p.bind(
            *operands, out_avals=tuple(out_avals), in_names=tuple(all_in),
            out_names=tuple(out_names), lowering_input_output_aliases=(),
            sim_require_finite=True, sim_require_nnan=True, nc=nc))

    devices = jax.devices()[:n_cores]
    mesh = Mesh(np.asarray(devices), ("core",))
    nio = n_params + len(out_names)
    sm_kw = dict(mesh=mesh, in_specs=(PartitionSpec("core"),) * nio,
                 out_specs=(PartitionSpec("core"),) * len(out_names))
    try:
        smapped = shard_map(_body, check_vma=False, **sm_kw)
    except TypeError:
        smapped = shard_map(_body, check_rep=False, **sm_kw)
    sharded = jax.jit(smapped, donate_argnums=donate, keep_unused=True)
    sh = NamedSharding(mesh, PartitionSpec("core"))
    zeros = jax.jit(
        lambda: tuple(jnp.zeros((n_cores * a.shape[0], *a.shape[1:]), a.dtype)
                      for a in out_avals),
        out_shardings=(sh,) * len(out_avals))()
    concat_in = [np.concatenate([np.asarray(m[k]) for m in in_maps], axis=0)
                 for k in in_names]
    outs = [np.asarray(o) for o in sharded(*concat_in, *zeros)]
    per_core = []
    for c in range(n_cores):
        d = {}
        for name, arr in zip(out_names, outs):
            s0 = arr.shape[0] // n_cores
            d[name] = arr[c * s0:(c + 1) * s0]
        per_core.append(d)
    return per_core


def kernel(x, w_q, w_k, w_v, freqs_cos, freqs_sin, _want_results=False, _trace=False):
    if "nc" not in _CACHE:
        _CACHE["nc"] = _build()
    nc = _CACHE["nc"]
    in_maps = _prep_inputs(np.asarray(x, np.float32), np.asarray(w_q, np.float32),
                           np.asarray(w_k, np.float32), np.asarray(w_v, np.float32),
                           np.asarray(freqs_cos, np.float32),
                           np.asarray(freqs_sin, np.float32))
    if _trace:
        kr = run_bass_kernel_spmd(nc, in_maps, core_ids=list(range(8)), trace=True)
        out = _assemble(kr.results)
        return (out, kr) if _want_results else out
    try:
        results = _run_pjrt(nc, in_maps)
    except Exception as e:
        print(f"kernel: _run_pjrt failed ({type(e).__name__}: {e}); "
              "falling back to run_bass_kernel_spmd", file=sys.stderr)
        kr = run_bass_kernel_spmd(nc, in_maps, core_ids=list(range(8)))
        results = kr.results
    out = _assemble(results)
    if _want_results:
        return out, results
    return out


# revision 70
# speedup vs baseline: 1.2472x; 1.1923x over previous
"""Causal single-head attention (B=2, S=4096, D=1024) + RoPE on 8 TRN2 cores.

Hybrid-gather design: cores 4b+i (b=batch, i=rank 0..3) each receive the FULL
weights and the full x^T of their batch from the host. Each core:

  1. projects+ropes its 1/4 shard of the kv TAIL (blocks 5..7, 384 rows) and
     AllGathers K^T|V within its batch group — the collective launches at
     ~37us and hides completely behind the local prefix work,
  2. projects K^T for the kv PREFIX (blocks 0..4) redundantly, directly in
     transposed layout (stationary=W^T, moving=x^T), roped in that layout,
     with its own 8 q-chunks QCH[i] (balanced causal mass) along the way,
  3. projects V for the prefix blocks, interleaved with
  4. 8 attention slots: slot t attends the first 512*(t+1) kv columns; scores
     are computed TRANSPOSED (S^T[kv,q]) as four 128-col accumulation chains
     per [C,512] PSUM quartet tile so exp output IS P^T (no PE transposes);
     the causal edge mask is folded into the PSUM accumulation per-slice
     (stationary=identity, moving=host-built mask); softmax runs without
     max-subtraction (|scores*scale| <= ~3, f32-exact) with row sums via
     ones-matmul chains.

Output: final softmax-normalized rows (bf16); host scatters them.
"""

import sys

sys.path.insert(0, "/opt/trn_rl_repo")

import math
from contextlib import ExitStack

import ml_dtypes
import numpy as np

import concourse.bass as bass
import concourse.tile as tile
from concourse import bacc, mybir
from concourse.bass_utils import run_bass_kernel_spmd
from concourse.masks import make_identity

BF16 = mybir.dt.bfloat16
F32 = mybir.dt.float32
NPBF16 = ml_dtypes.bfloat16

B, S, D = 2, 4096, 1024
H = D // 2
C = 128
NQC = S // C                  # 32 chunks of 128 rows
NOC = 8                       # own q chunks per core
NOR = NOC * C                 # 1024 own q rows
NBLK = S // 512               # 8 512-row blocks
SCALE = 1.0 / math.sqrt(D)
NEG = -30000.0

QCH = [sorted([4 * t + i for t in range(4)] + [4 * t + 3 - i for t in range(4, 8)])
       for i in range(4)]

# kv blocks [MLOC, 8) are projected 1/4-sharded per rank and AllGathered
# within the batch group (the 172us collective hides fully behind the local
# prefix work); blocks [0, MLOC) are projected redundantly on every core.
MLOC = 5                      # locally-projected prefix blocks
SH0 = MLOC * 512              # first gathered row
SHR = (NBLK - MLOC) * 512 // 4   # 384 shard rows per rank
GROUPS = [[0, 1, 2, 3], [4, 5, 6, 7]]

_CACHE = {}


def _emit_qchunk(nc, qstr_p, qraw_p, ktmp_p, qps_p, csq, wq_sb, xq, qt_sb, blk):
    """Project (transposed layout) + rope own q chunk #blk (streamed x)."""
    qcols = slice(blk * C, (blk + 1) * C)
    cq = qstr_p.tile([C, 8, C], BF16, tag="cq", name=f"cq_{blk}")
    nc.sync.dma_start(cq[:], csq[:, :, qcols])
    xqc = qstr_p.tile([C, 8, C], BF16, tag="xqc", name=f"xqc_{blk}")
    nc.sync.dma_start(xqc[:], xq[:, :, qcols])
    qraw_c = qraw_p.tile([C, 8, C], BF16, tag="qraw", name=f"qraw_{blk}")
    for ec in range(8):
        qp = qps_p.tile([C, C], F32, tag="qps", name=f"qp_{blk}_{ec}")
        for dc in range(8):
            nc.tensor.matmul(qp[:], wq_sb[:, dc, ec * C:(ec + 1) * C],
                             xqc[:, dc, :],
                             start=(dc == 0), stop=(dc == 7))
        nc.scalar.copy(qraw_c[:, ec, :], qp[:])
    for pr in range(4):
        cc, ss = cq[:, pr, :], cq[:, pr + 4, :]
        re, im = qraw_c[:, pr, :], qraw_c[:, pr + 4, :]
        t0 = ktmp_p.tile([C, C], BF16, tag="qt0", name=f"qt0_{blk}_{pr}")
        t1 = ktmp_p.tile([C, C], BF16, tag="qt1", name=f"qt1_{blk}_{pr}")
        nc.vector.tensor_mul(t0[:], re, cc)
        nc.vector.tensor_mul(t1[:], im, ss)
        nc.vector.tensor_sub(qt_sb[:, pr, qcols], t0[:], t1[:])
        t2 = ktmp_p.tile([C, C], BF16, tag="qt2", name=f"qt2_{blk}_{pr}")
        t3 = ktmp_p.tile([C, C], BF16, tag="qt3", name=f"qt3_{blk}_{pr}")
        nc.vector.tensor_mul(t2[:], re, ss)
        nc.vector.tensor_mul(t3[:], im, cc)
        nc.vector.tensor_add(qt_sb[:, pr + 4, qcols], t2[:], t3[:])


def _build():
    nc = bacc.Bacc("TRN2", target_bir_lowering=False, debug=False,
                   enable_asserts=False, num_devices=8)

    xt = nc.dram_tensor("xt", [C, 8, S], BF16, kind="ExternalInput").ap()
    xq = nc.dram_tensor("xq", [C, 8, NOR], BF16, kind="ExternalInput").ap()
    csq = nc.dram_tensor("csq", [C, 8, NOR], BF16, kind="ExternalInput").ap()
    csk = nc.dram_tensor("csk", [C, 8, S], BF16, kind="ExternalInput").ap()
    w_in = nc.dram_tensor("w_in", [C, 24, D], BF16, kind="ExternalInput").ap()
    masks = nc.dram_tensor("masks", [C, 2, 512], BF16, kind="ExternalInput").ap()
    # own kv shard (rows SH0 + rank*SHR, host-sliced) + its rope tables
    xsh = nc.dram_tensor("xsh", [C, 8, SHR], BF16, kind="ExternalInput").ap()
    cssh = nc.dram_tensor("cssh", [C, 8, SHR], BF16, kind="ExternalInput").ap()

    o_fin = nc.dram_tensor("o_fin", [NOC, C, D], BF16, kind="ExternalOutput").ap()

    with tile.TileContext(nc) as tc, ExitStack() as ctx:
        const_p = ctx.enter_context(tc.tile_pool(name="const", bufs=1))
        ident = const_p.tile([C, C], BF16)
        make_identity(nc, ident[:])
        masks_sb = const_p.tile([C, 2, 512], BF16)
        ones_sb = const_p.tile([C, 1], BF16)
        nc.vector.memset(ones_sb[:], 1.0)

        # PE warmup: ~11us of dummy transposes while the first weight/x DMAs
        # stream, so the tensor engine is at full p-state when real matmuls
        # start (the results are never read). Source is a fast DVE memset
        # tile so warmup starts before the Pool-built identity is ready.
        with tc.tile_pool(name="wup", bufs=1, space="PSUM") as wup_p, \
             tc.tile_pool(name="wsrc", bufs=1) as wsrc_p:
            wsrc = wsrc_p.tile([C, C], BF16, tag="wsrc")
            nc.vector.memset(wsrc[:], 0.0)
            wup = wup_p.tile([C, C], BF16, tag="wup")
            for r in range(34):
                nc.tensor.transpose(wup[:], wsrc[:], wsrc[:])

        qt_p = ctx.enter_context(tc.tile_pool(name="qt", bufs=1))
        qt_sb = qt_p.tile([C, 8, NOR], BF16, tag="qt")
        # resident until the end: K^T and the V-projection weights (prefetched)
        kt_p = ctx.enter_context(tc.tile_pool(name="kt", bufs=1))
        kt_sb = kt_p.tile([C, 8, S], BF16, tag="kt")
        wv_sb = kt_p.tile([C, 8, D], BF16, tag="wv")

        dram_p = ctx.enter_context(tc.tile_pool(name="dram", bufs=1, space="DRAM"))
        kvb_in = dram_p.tile([C, 2 * 8 * SHR], BF16)
        kvb_out = dram_p.tile([4, C, 2 * 8 * SHR], BF16)

        # ---- phase 1: own kv shard (rows SH0+rank*SHR) projected+roped and
        #      AllGathered within the group; then streamed K^T projection+rope
        #      for the local prefix blocks with own-q chunks along the way ----
        with tc.tile_pool(name="ph1", bufs=1) as p1_p, \
             tc.tile_pool(name="kstr", bufs=2) as kstr_p, \
             tc.tile_pool(name="cstr", bufs=2) as cstr_p, \
             tc.tile_pool(name="qstr", bufs=2) as qstr_p, \
             tc.tile_pool(name="kraw", bufs=4) as kraw_p, \
             tc.tile_pool(name="qraw", bufs=2) as qraw_p, \
             tc.tile_pool(name="ktmp", bufs=2) as ktmp_p, \
             tc.tile_pool(name="kps", bufs=4, space="PSUM") as kps_p, \
             tc.tile_pool(name="qps", bufs=3, space="PSUM") as qps_p:
            # DMA issue order = order of first use: the shard projection only
            # needs wk+xsh (~7us of DMA); everything else streams behind.
            # per-dc interleaved weight/x loads: the shard chain's accum step
            # dc only needs its own dc-slices, so real matmuls start ~3us in,
            # DMA-paced (subtile deps stage it automatically)
            wk_sb = p1_p.tile([C, 8, D], BF16, tag="wk")
            with tc.tile_pool(name="shp", bufs=1) as sh_p:
                xsh_sb = kstr_p.tile([C, 8, SHR], BF16, tag="xb", name="xsh")
                for dcd in range(8):
                    nc.sync.dma_start(wk_sb[:, dcd, :], w_in[:, 8 + dcd, :])
                    nc.sync.dma_start(xsh_sb[:, dcd, :], xsh[:, dcd, :])
                cssh_sb = cstr_p.tile([C, 8, SHR], BF16, tag="cb", name="cssh")
                nc.sync.dma_start(cssh_sb[:], cssh)
                nc.sync.dma_start(wv_sb[:], w_in[:, 16:24, :])
                xb0 = kstr_p.tile([C, 8, 512], BF16, tag="xb", name="xb_0")
                nc.sync.dma_start(xb0[:], xt[:, :, 0:512])
                cb0 = cstr_p.tile([C, 8, 512], BF16, tag="cb", name="cb_0")
                nc.sync.dma_start(cb0[:], csk[:, :, 0:512])
                wq_sb = p1_p.tile([C, 8, D], BF16, tag="wq")
                nc.sync.dma_start(wq_sb[:], w_in[:, 0:8, :])
                # shard K^T projection + rope -> ktsh_sb [C, 8*SHR]
                ktsh_sb = sh_p.tile([C, 8 * SHR], BF16, tag="ktsh")
                for pr in range(4):
                    kraw = []
                    for dc in (pr, pr + 4):
                        ps = kps_p.tile([C, SHR], F32, tag="kps",
                                        name=f"kpssh_{dc}")
                        for dcd in range(8):
                            nc.tensor.matmul(ps[:],
                                             wk_sb[:, dcd, dc * C:(dc + 1) * C],
                                             xsh_sb[:, dcd, :],
                                             start=(dcd == 0), stop=(dcd == 7))
                        kr = kraw_p.tile([C, SHR], BF16, tag="kraw",
                                         name=f"krawsh_{dc}")
                        nc.scalar.copy(kr[:], ps[:])
                        kraw.append(kr)
                    re, im = kraw[0], kraw[1]
                    cc, ss = cssh_sb[:, pr, :], cssh_sb[:, pr + 4, :]
                    t0 = ktmp_p.tile([C, SHR], BF16, tag="kt0", name=f"ksht0_{pr}")
                    t1 = ktmp_p.tile([C, SHR], BF16, tag="kt1", name=f"ksht1_{pr}")
                    nc.vector.tensor_mul(t0[:], re[:], cc)
                    nc.vector.tensor_mul(t1[:], im[:], ss)
                    nc.vector.tensor_sub(ktsh_sb[:, pr * SHR:(pr + 1) * SHR],
                                         t0[:], t1[:])
                    t2 = ktmp_p.tile([C, SHR], BF16, tag="kt2", name=f"ksht2_{pr}")
                    t3 = ktmp_p.tile([C, SHR], BF16, tag="kt3", name=f"ksht3_{pr}")
                    nc.vector.tensor_mul(t2[:], re[:], ss)
                    nc.vector.tensor_mul(t3[:], im[:], cc)
                    nc.vector.tensor_add(
                        ktsh_sb[:, (pr + 4) * SHR:(pr + 5) * SHR], t2[:], t3[:])
                nc.sync.dma_start(kvb_in[:, 0:8 * SHR], ktsh_sb[:])
                # shard V projection -> vsh_sb [C, 3*D]
                vsh_sb = sh_p.tile([C, (SHR // C) * D], BF16, tag="vsh")
                for c4 in range(SHR // C):
                    for h in range(2):
                        vp = kps_p.tile([C, 512], F32, tag="kps",
                                        name=f"vpssh_{c4}_{h}")
                        for dc in range(8):
                            nc.tensor.matmul(
                                vp[:], xsh_sb[:, dc, c4 * C:(c4 + 1) * C],
                                wv_sb[:, dc, h * 512:(h + 1) * 512],
                                start=(dc == 0), stop=(dc == 7))
                        nc.scalar.copy(
                            vsh_sb[:, c4 * D + h * 512:c4 * D + (h + 1) * 512],
                            vp[:])
                nc.sync.dma_start(kvb_in[:, 8 * SHR:16 * SHR], vsh_sb[:])
                nc.gpsimd.collective_compute(
                    "AllGather", mybir.AluOpType.bypass, replica_groups=GROUPS,
                    ins=[kvb_in[:].opt()], outs=[kvb_out[:].opt()])
                # gathered K^T tail -> kt_sb columns [SH0, S)
                for r in range(4):
                    for dc in range(8):
                        nc.gpsimd.dma_start(
                            kt_sb[:, dc, SH0 + r * SHR:SH0 + (r + 1) * SHR],
                            kvb_out[r, :, dc * SHR:(dc + 1) * SHR])

            for blk in range(MLOC):
                if blk == 1:
                    nc.sync.dma_start(xq_sb[:, :, 2 * C:NOR], xq[:, :, 2 * C:NOR])
                if blk == 2:
                    nc.sync.dma_start(masks_sb[:], masks)
                rows = slice(blk * 512, (blk + 1) * 512)
                if blk == 0:
                    xb, cb = xb0, cb0
                else:
                    xb = kstr_p.tile([C, 8, 512], BF16, tag="xb", name=f"xb_{blk}")
                    nc.sync.dma_start(xb[:], xt[:, :, rows])
                    cb = cstr_p.tile([C, 8, 512], BF16, tag="cb", name=f"cb_{blk}")
                    nc.sync.dma_start(cb[:], csk[:, :, rows])
                for pr in range(4):
                    kraw = []
                    for dc in (pr, pr + 4):
                        ps = kps_p.tile([C, 512], F32, tag="kps",
                                        name=f"kps_{blk}_{dc}")
                        for dcd in range(8):
                            nc.tensor.matmul(ps[:],
                                             wk_sb[:, dcd, dc * C:(dc + 1) * C],
                                             xb[:, dcd, :],
                                             start=(dcd == 0), stop=(dcd == 7))
                        kr = kraw_p.tile([C, 512], BF16, tag="kraw",
                                         name=f"kraw_{blk}_{dc}")
                        nc.scalar.copy(kr[:], ps[:])
                        kraw.append(kr)
                    re, im = kraw[0], kraw[1]
                    cc, ss = cb[:, pr, :], cb[:, pr + 4, :]
                    t0 = ktmp_p.tile([C, 512], BF16, tag="kt0", name=f"kt0_{blk}_{pr}")
                    t1 = ktmp_p.tile([C, 512], BF16, tag="kt1", name=f"kt1_{blk}_{pr}")
                    nc.vector.tensor_mul(t0[:], re[:], cc)
                    nc.vector.tensor_mul(t1[:], im[:], ss)
                    nc.vector.tensor_sub(kt_sb[:, pr, rows], t0[:], t1[:])
                    t2 = ktmp_p.tile([C, 512], BF16, tag="kt2", name=f"kt2_{blk}_{pr}")
                    t3 = ktmp_p.tile([C, 512], BF16, tag="kt3", name=f"kt3_{blk}_{pr}")
                    nc.vector.tensor_mul(t2[:], re[:], ss)
                    nc.vector.tensor_mul(t3[:], im[:], cc)
                    nc.vector.tensor_add(kt_sb[:, pr + 4, rows], t2[:], t3[:])

                _emit_qchunk(nc, qstr_p, qraw_p, ktmp_p, qps_p, csq,
                             wq_sb, xq_sb, qt_sb, blk)
            for blk in range(MLOC, NBLK):
                _emit_qchunk(nc, qstr_p, qraw_p, ktmp_p, qps_p, csq,
                             wq_sb, xq_sb, qt_sb, blk)

        # ---- phase V + attention, interleaved per 512-block/slot ----
        v_p = ctx.enter_context(tc.tile_pool(name="v", bufs=1))
        v_sb = v_p.tile([C, NQC, D], BF16, tag="v")
        # gathered V tail -> v_sb chunks [SH0/C, 32)
        for r in range(4):
            for c4 in range(SHR // C):
                nc.gpsimd.dma_start(
                    v_sb[:, SH0 // C + r * (SHR // C) + c4, :],
                    kvb_out[r, :, 8 * SHR + c4 * D:8 * SHR + (c4 + 1) * D])
        with tc.tile_pool(name="vstr", bufs=2) as vstr_p, \
             tc.tile_pool(name="pp", bufs=2) as pp_p, \
             tc.tile_pool(name="at", bufs=2) as at_p, \
             tc.tile_pool(name="vps", bufs=2, space="PSUM") as vps_p, \
             tc.tile_pool(name="scps", bufs=2, space="PSUM") as scps_p, \
             tc.tile_pool(name="ops", bufs=2, space="PSUM") as ops_p, \
             tc.tile_pool(name="lsps", bufs=2, space="PSUM") as lsps_p:

            def _emit_slot(t):
                # Scores computed TRANSPOSED (S^T[kv, q]) into 128-col slices
                # of a [C,512] PSUM quartet tile: exp output IS P^T (no PE
                # transposes), row sums via ones-matmul chains, mask appended
                # per-slice on the diagonal quartet.
                kpat = 0 if t < 4 else 1
                qc = slice(t * C, (t + 1) * C)
                p_sb = pp_p.tile([C, S], BF16, tag="p", name=f"p_{t}")
                ls_ps = lsps_p.tile([C, 1], F32, tag="lsps", name=f"ls_{t}")
                for u in range(t + 1):
                    cols = slice(u * 512, (u + 1) * 512)
                    sps = scps_p.tile([C, 512], F32, tag="scps",
                                      name=f"sps_{t}_{u}")
                    for c in range(4):
                        ch = 4 * u + c
                        csl = slice(c * C, (c + 1) * C)
                        diag = u == t
                        for dc in range(8):
                            nc.tensor.matmul(
                                sps[:, csl],
                                kt_sb[:, dc, ch * C:(ch + 1) * C],
                                qt_sb[:, dc, qc],
                                start=(dc == 0),
                                stop=(dc == 7 and not diag))
                        if diag:
                            nc.tensor.matmul(sps[:, csl], ident[:],
                                             masks_sb[:, kpat, csl],
                                             start=False, stop=True)
                    nc.scalar.activation(p_sb[:, cols], sps[:],
                                         mybir.ActivationFunctionType.Exp,
                                         scale=SCALE)
                    # lsum for quartet u-1 (exp already done -> no PE stall);
                    # quartet t's lsum lands after the loop
                    for ud in ([u - 1] if u >= 1 else []) + ([t] if u == t else []):
                        for c in range(4):
                            ch = 4 * ud + c
                            nc.tensor.matmul(ls_ps[:],
                                             p_sb[:, ch * C:(ch + 1) * C],
                                             ones_sb[:],
                                             start=(ud == 0 and c == 0),
                                             stop=(ud == t and c == 3))
                rinv = at_p.tile([C, 1], F32, tag="rinv", name=f"rinv_{t}")
                nc.vector.reciprocal(rinv[:], ls_ps[:])

                ob = at_p.tile([C, D], BF16, tag="ob", name=f"ob_{t}")
                # final slot: tapered output pieces shorten the
                # end-of-kernel scale+store chain
                widths = [256, 256, 256, 128, 128] if t == NOC - 1 else [512, 512]
                off = 0
                for h, qw in enumerate(widths):
                    cols = slice(off, off + qw)
                    off += qw
                    o_ps = ops_p.tile([C, qw], F32, tag="ops",
                                      name=f"ops_{t}_{h}")
                    for u in range(t + 1):
                        for j in range(4):
                            ch = 4 * u + j
                            nc.tensor.matmul(o_ps[:],
                                             p_sb[:, ch * C:(ch + 1) * C],
                                             v_sb[:, ch, cols],
                                             start=(u == 0 and j == 0),
                                             stop=(u == t and j == 3))
                    if h % 2 == 0:
                        nc.vector.tensor_scalar_mul(ob[:, cols], o_ps[:],
                                                    rinv[:])
                    else:
                        nc.scalar.mul(ob[:, cols], o_ps[:], rinv[:])
                    nc.sync.dma_start(o_fin[t, :, cols], ob[:, cols])

            for t in range(NOC):
                # V projection for kv block t (chunks 4t..4t+3); the tail
                # blocks [MLOC, 8) arrive via the AllGather instead
                if t < MLOC:
                    rows = slice(t * 512, (t + 1) * 512)
                    xb = vstr_p.tile([C, 8, 512], BF16, tag="vxb",
                                     name=f"vxb_{t}")
                    nc.sync.dma_start(xb[:], xt[:, :, rows])
                    for c4 in range(4):
                        rsl = slice(c4 * C, (c4 + 1) * C)
                        for h in range(2):
                            cols = slice(h * 512, (h + 1) * 512)
                            vp = vps_p.tile([C, 512], F32, tag="vps",
                                            name=f"vps_{t}_{c4}_{h}")
                            for dc in range(8):
                                nc.tensor.matmul(vp[:], xb[:, dc, rsl],
                                                 wv_sb[:, dc, cols],
                                                 start=(dc == 0), stop=(dc == 7))
                            nc.scalar.copy(v_sb[:, 4 * t + c4, cols], vp[:])

                # attention slot t: q chunk QCH[i][t], kv cols [0, 512*(t+1))
                _emit_slot(t)

    nc.compile()
    return nc


def _xt_blocked(rows_x):
    """[n, D] float -> [C, 8, n] bf16 blocked transpose."""
    return np.ascontiguousarray(
        rows_x.astype(NPBF16).reshape(-1, 8, C).transpose(2, 1, 0))


def _masks(i):
    # transposed-score masks: tile [kv(part) p, q j] allows kv<=q -> p<=j
    tri = np.where(np.arange(C)[:, None] <= np.arange(C)[None, :], 0.0, NEG)
    m = np.zeros((C, 2, 512), np.float32)
    for k, diag in enumerate((i, 3 - i)):
        for c in range(4):
            if c > diag:
                m[:, k, c * C:(c + 1) * C] = NEG
            elif c == diag:
                m[:, k, c * C:(c + 1) * C] = tri
    return np.ascontiguousarray(m.astype(NPBF16))


def _prep_inputs(x, w_q, w_k, w_v, freqs_cos, freqs_sin):
    perm = np.concatenate([np.arange(0, D, 2), np.arange(1, D, 2)])
    wqT = np.ascontiguousarray(w_q[perm, :].T)
    wkT = np.ascontiguousarray(w_k[perm, :].T)
    wvT = np.ascontiguousarray(w_v.T)

    def blk(wt):  # [D, D] -> [C, dc, e]
        return wt.astype(NPBF16).reshape(8, C, D).transpose(1, 0, 2)

    flat24 = np.ascontiguousarray(
        np.concatenate([blk(wqT), blk(wkT), blk(wvT)], axis=1))  # [C, 24, D]

    cs_all = np.concatenate([freqs_cos, freqs_sin], axis=1)  # [S, D]
    csk_b = _xt_blocked(cs_all)                              # same for all cores
    xt_b = [_xt_blocked(np.asarray(x[b])) for b in range(B)]

    in_maps = []
    for core in range(8):
        b, i = divmod(core, 4)
        qrows = (np.asarray(QCH[i])[:, None] * C + np.arange(C)[None, :]).reshape(-1)
        shrows = np.arange(SH0 + i * SHR, SH0 + (i + 1) * SHR)
        in_maps.append({
            "xt": xt_b[b],
            "xq": _xt_blocked(np.asarray(x[b])[qrows]),
            "csq": _xt_blocked(cs_all[qrows]),
            "csk": csk_b,
            "w_in": flat24,
            "masks": _masks(i),
            "xsh": _xt_blocked(np.asarray(x[b])[shrows]),
            "cssh": _xt_blocked(cs_all[shrows]),
        })
    return in_maps


def _assemble(results):
    out = np.empty((B, S, D), np.float32)
    for core in range(8):
        b, i = divmod(core, 4)
        o = np.asarray(results[core]["o_fin"], np.float32)  # [NOC, C, D]
        for t, j in enumerate(QCH[i]):
            out[b, j * C:(j + 1) * C] = o[t]
    return out


def _run_pjrt(nc, in_maps, n_cores=8):
    """Like bass2jax.run_bass_via_pjrt, but creates the donated output
    buffers ON DEVICE (jit zeros) instead of uploading host zeros."""
    import jax
    import jax.numpy as jnp
    from jax.sharding import Mesh, NamedSharding, PartitionSpec
    try:
        from jax import shard_map
    except ImportError:
        from jax.experimental.shard_map import shard_map
    from concourse.bass2jax import (_bass_exec_p, install_neuronx_cc_hook,
                                    partition_id_tensor)

    install_neuronx_cc_hook()
    partition_name = nc.partition_id_tensor.name if nc.partition_id_tensor else None
    in_names, out_names, out_avals = [], [], []
    for alloc in nc.m.functions[0].allocations:
        if not isinstance(alloc, mybir.MemoryLocationSet):
            continue
        name = alloc.memorylocations[0].name
        if alloc.kind == "ExternalInput":
            if name != partition_name:
                in_names.append(name)
        elif alloc.kind == "ExternalOutput":
            out_avals.append(jax.core.ShapedArray(
                tuple(alloc.tensor_shape), mybir.dt.np(alloc.dtype)))
            out_names.append(name)
    n_params = len(in_names)
    all_in = list(in_names) + list(out_names)
    if partition_name is not None:
        all_in.append(partition_name)
    donate = tuple(range(n_params, n_params + len(out_names)))

    def _body(*args):
        operands = list(args)
        if partition_name is not None:
            operands.append(partition_id_tensor())
        return tuple(_bass_exec_p.bind(
            *operands, out_avals=tuple(out_avals), in_names=tuple(all_in),
            out_names=tuple(out_names), lowering_input_output_aliases=(),
            sim_require_finite=True, sim_require_nnan=True, nc=nc))

    devices = jax.devices()[:n_cores]
    mesh = Mesh(np.asarray(devices), ("core",))
    nio = n_params + len(out_names)
    sm_kw = dict(mesh=mesh, in_specs=(PartitionSpec("core"),) * nio,
                 out_specs=(PartitionSpec("core"),) * len(out_names))
    try:
        smapped = shard_map(_body, check_vma=False, **sm_kw)
    except TypeError:
        smapped = shard_map(_body, check_rep=False, **sm_kw)
    sharded = jax.jit(smapped, donate_argnums=donate, keep_unused=True)
    sh = NamedSharding(mesh, PartitionSpec("core"))
    zeros = jax.jit(
        lambda: tuple(jnp.zeros((n_cores * a.shape[0], *a.shape[1:]), a.dtype)
                      for a in out_avals),
        out_shardings=(sh,) * len(out_avals))()
    concat_in = [np.concatenate([np.asarray(m[k]) for m in in_maps], axis=0)
                 for k in in_names]
    outs = [np.asarray(o) for o in sharded(*concat_in, *zeros)]
    per_core = []
    for c in range(n_cores):
        d = {}
        for name, arr in zip(out_names, outs):
            s0 = arr.shape[0] // n_cores
            d[name] = arr[c * s0:(c + 1) * s0]
        per_core.append(d)
    return per_core


def kernel(x, w_q, w_k, w_v, freqs_cos, freqs_sin, _want_results=False, _trace=False):
    if "nc" not in _CACHE:
        _CACHE["nc"] = _build()
    nc = _CACHE["nc"]
    in_maps = _prep_inputs(np.asarray(x, np.float32), np.asarray(w_q, np.float32),
                           np.asarray(w_k, np.float32), np.asarray(w_v, np.float32),
                           np.asarray(freqs_cos, np.float32),
                           np.asarray(freqs_sin, np.float32))
    if _trace:
        kr = run_bass_kernel_spmd(nc, in_maps, core_ids=list(range(8)), trace=True)
        out = _assemble(kr.results)
        return (out, kr) if _want_results else out
    try:
        results = _run_pjrt(nc, in_maps)
    except Exception as e:
        print(f"kernel: _run_pjrt failed ({type(e).__name__}: {e}); "
              "falling back to run_bass_kernel_spmd", file=sys.stderr)
        kr = run_bass_kernel_spmd(nc, in_maps, core_ids=list(range(8)))
        results = kr.results
    out = _assemble(results)
    if _want_results:
        return out, results
    return out
